# revision 18
# baseline (speedup 1.0000x reference)
"""BayesianAttention (ALiBi-style power-law prior + causal mask) on 8 trn2 cores.

Self-contained: builds a Bass/Tile kernel, shards heads across 8 NeuronCores
(2 heads per core; wq/wk/wv column-sharded, wo row-sharded), runs via a
shard_map'd bass program, and reduces the partial outputs on host.

v2 layout (all-bf16 operands, batched DMA, software-pipelined attention):
  host sends x^T [c, i] bf16; device computes q^T/k^T/v^T = W^T x^T in two
  PSUM passes (q,k then v), transposes v, s^T[j,i] = k^T_j . q^T_i,
  probs = exp(s^T) * EG  (EG = exp(prior + mask), a Toeplitz table indexed
  by j - i, precomputed on host, 0 where masked), o^T[d,i] = v^T probs with
  softmax sum via an all-ones stationary matmul, out^T[e,i] = wo^T (o^T/sum).
  Host returns sum_c(out^T_c)^T.

  All DMAs are batched (few large transfers) and issued on the SP queue;
  the attention inner loop is one global software pipeline across (ib, h)
  blocks so the PE never drains between blocks; phase-3 output rows are
  accumulated in SBUF and written once per 128-row stripe.
"""

import math
import os
from collections import deque

import ml_dtypes
import numpy as np

S = 2048          # sequence length
DIM = 2048        # model dim
H = 16            # heads
HD = 128          # head dim
N_CORES = 8
HL = H // N_CORES  # heads per core (2)
DL = HL * HD       # local projected dim (256)
IB = 512           # i-block (query block, moving free dim)
NIB = S // IB
NJT = S // 128     # key tiles of 128
NQ = 4             # x row-quads (512 rows each)
EPS = 1e-5
MASKED_THRESH = -1e8   # additive mask values below this mean "fully masked"

MM_DTYPE = os.environ.get("KBA_DTYPE", "bf16")  # "bf16" | "f32"
LAG = int(os.environ.get("KBA_LAG", "5"))  # scores->o-matmul emission lag

LAST_RUN_INFO = {}


# ---------------------------------------------------------------- tile patch
def _apply_tile_patch():
    """walrus CoreV3 codegen tolerates only one sync-wait on an InstDrain;
    the tile-exit drain waits on the whole global clock. Spread the waits
    across extra SP nops."""
    import concourse.tile as tile
    from concourse import mybir
    from concourse.vector_clock import ScopedClock

    if getattr(tile.TileContext, "_kba_patched", False):
        return

    def _drain_and_barrier(self, tick_clock, wait_clock):
        nc = self.nc
        drain_inst = nc.sync.drain()
        wait_clock.add_sem_waits(
            drain_inst.ins, ScopedClock({None: tick_clock.global_clock})
        )
        si = drain_inst.ins.sync_info
        waits = list(si.on_wait or [])
        if len(waits) > 1:
            si.on_wait = waits[:1]
            for i in range(1, len(waits)):
                nop = nc.sync.nop(nofuse=True)
                nop.ins.sync_info = mybir.SyncInfo(
                    on_wait=waits[i : i + 1], on_update=[]
                )
        nc.all_engine_barrier()
        assert self.sems is not None
        popped = nc._tile_sem_poison_stack.pop()
        assert popped is self._sem_poison
        nc.clear_and_free_semaphores(list(self.sems.allocated().values()))
        nc.all_engine_barrier()

    tile.TileContext._drain_and_barrier = _drain_and_barrier
    tile.TileContext._kba_patched = True

    try:
        import concourse.tile_utils as tile_utils

        tile_utils.max_sbuf_usage = 208 * 1024
    except Exception:
        pass


# ------------------------------------------------------------- host helpers
def _toeplitz_profile(m2):
    """If mask[i, j] == phi(j - i) for all i,j, return phi (length 2S-1,
    index t + S - 1), else None."""
    phi = np.empty(2 * S - 1, dtype=np.float32)
    phi[S - 1 :] = m2[0, :]
    phi[: S - 1] = m2[1:, 0][::-1]
    idx = (np.arange(S)[None, :] - np.arange(S)[:, None]) + (S - 1)
    if np.array_equal(phi[idx], m2):
        return phi
    return None


# tiles/columns whose max prior weight is below this contribute nothing:
# host dry-run shows even element-granularity dropping at 1e-4 leaves the
# output bit-identical (errors move only at 1e-3)
EG_TAU = 1e-4


def _eg_profile(head, shape, scale, loc, start_pos, phi):
    """1-D prior weight profile eg1[d + (S-1)] = exp(prior(d) + phi(-d)) for
    d = i - j in [-(S-1), S-1]. The 2-D EG table is eg1 evaluated per (p,u);
    a tile's max EG is the max of eg1 over the tile's contiguous d-range."""
    d = np.arange(-(S - 1), S, dtype=np.int64)
    dist = (-d - start_pos).astype(np.float32)
    sh = np.float32(shape[0, head, 0, 0])
    sc = np.float32(scale[0, head, 0, 0])
    lo = np.float32(loc[0, head, 0, 0])
    loc_t = np.float32(np.exp(lo) - np.exp(-lo))
    z = (dist - loc_t) * np.exp(sc, dtype=np.float32)
    g = -np.power(np.abs(z) + np.float32(EPS), sh, dtype=np.float32)
    g = g + phi[np.clip(-d + (S - 1), 0, 2 * S - 2)]
    return np.exp(g)


def _head_kept_tiles(eg1):
    """kept[ib] = j-tiles where the head's prior x mask weight is non-negligible
    somewhere in the [IB x 128] block."""
    kept = []
    for ib in range(NIB):
        row = []
        for jt in range(NJT):
            dlo = ib * IB - jt * 128 - 127
            dhi = ib * IB + IB - 1 - jt * 128
            lo = max(dlo + (S - 1), 0)
            hi = min(dhi + (S - 1), 2 * S - 2)
            if lo <= hi and eg1[lo : hi + 1].max() >= EG_TAU:
                row.append(jt)
        kept.append(row)
    return kept


def _tile_support(eg_slot, ib, jt):
    """(off, w): the i-column range of block ib where tile jt has any
    non-negligible prior x mask weight. eg_slot is the per-slot max of the
    heads' 1-D profiles; column i_local covers d = (ib*IB + i_local) - j for
    j in the tile, a 128-wide window of eg_slot."""
    pad = np.zeros(2 * S - 1 + 256, np.float32)
    pad[128 : 128 + 2 * S - 1] = eg_slot
    # max over the 128-wide window ending at d = ib*IB + i - jt*128
    idx = (ib * IB + np.arange(IB) - jt * 128) + (S - 1) + 128
    win = np.lib.stride_tricks.sliding_window_view(pad, 128)
    colmax = win[idx - 127].max(axis=1)
    on = np.nonzero(colmax >= EG_TAU)[0]
    assert on.size, "kept tile with empty support"
    off = int(on[0]) & ~3
    end = min(IB, (int(on[-1]) + 1 + 3) & ~3)
    return off, end - off


def _eg_geometry(kept_slots):
    """The EG table covers u = base..base+IB for every kept (ib, jt), where
    base = ib*IB - jt*128 + (S-1). Returns (offset, width)."""
    bases = [
        ib * IB - jt * 128 + (S - 1)
        for kept in kept_slots
        for ib in range(NIB)
        for jt, _, _ in kept[ib]
    ]
    off = min(bases)
    width = max(bases) + IB - off
    return off, width


def _eg_table(head, shape, scale, loc, start_pos, phi, eg_off, eg_w):
    """[128, eg_w] float32: EG[p, u'] = exp(prior(d) + phi(-d)), 0 where
    masked/out of range, with d = i - j = (u' + eg_off) - p - (S - 1)."""
    p = np.arange(128, dtype=np.int64)[:, None]
    u = eg_off + np.arange(eg_w, dtype=np.int64)[None, :]
    d = u - p - (S - 1)          # i - j
    dist = (-d - start_pos).astype(np.float32)  # k_pos - q_pos
    sh = np.float32(shape[0, head, 0, 0])
    sc = np.float32(scale[0, head, 0, 0])
    lo = np.float32(loc[0, head, 0, 0])
    loc_t = np.float32(np.exp(lo) - np.exp(-lo))
    z = (dist - loc_t) * np.exp(sc, dtype=np.float32)
    g = -np.power(np.abs(z) + np.float32(EPS), sh, dtype=np.float32)
    t = np.clip(-d + (S - 1), 0, 2 * S - 2)
    g = g + phi[t]
    g[(-d < -(S - 1)) | (-d > (S - 1))] = -np.inf  # out of range: never read
    return np.ascontiguousarray(np.exp(g).astype(np.float32))


# ------------------------------------------------------------ program build
_PROGRAM_CACHE = {}


def _build_program(mm_name, kept_key, eg_off, eg_w, repeat=1):
    key = (mm_name, kept_key, eg_off, eg_w, repeat)
    if key in _PROGRAM_CACHE:
        return _PROGRAM_CACHE[key]

    import concourse.bass as bass
    import concourse.tile as tile
    from concourse import bacc, mybir
    from concourse.masks import make_identity

    _apply_tile_patch()

    f32 = mybir.dt.float32
    sdt = mybir.dt.bfloat16 if mm_name == "bf16" else f32

    # kept_key[h][ib] = j-tiles for local head slot h (slot 0 carries the
    # narrow-window heads, slot 1 the wide ones; identical across cores)
    kept = [[list(row) for row in slot] for slot in kept_key]

    nc = bacc.Bacc(
        "TRN2", target_bir_lowering=False, debug=False, num_devices=N_CORES
    )
    xT_d = nc.dram_tensor("xT", [S, S], sdt, kind="ExternalInput")
    wq_d = nc.dram_tensor("wq", [S, DL], sdt, kind="ExternalInput")
    wk_d = nc.dram_tensor("wk", [S, DL], sdt, kind="ExternalInput")
    wv_d = nc.dram_tensor("wv", [S, DL], sdt, kind="ExternalInput")
    wo_d = nc.dram_tensor("wo", [DL, S], sdt, kind="ExternalInput")
    eg_d = nc.dram_tensor("eg", [HL, 128, eg_w], sdt, kind="ExternalInput")
    outT_d = nc.dram_tensor("outT", [S, S], sdt, kind="ExternalOutput")

    Exp = mybir.ActivationFunctionType.Exp
    Copy = mybir.ActivationFunctionType.Copy

    with tile.TileContext(nc) as tc:
        import contextlib

        with contextlib.ExitStack() as ctx:
            consts = ctx.enter_context(tc.tile_pool(name="consts", bufs=1))
            persist = ctx.enter_context(tc.tile_pool(name="persist", bufs=1))
            xpool = ctx.enter_context(tc.tile_pool(name="xp", bufs=8))
            ppool = ctx.enter_context(tc.tile_pool(name="probs", bufs=4))
            vtpool = ctx.enter_context(tc.tile_pool(name="vt", bufs=2))
            rpool = ctx.enter_context(tc.tile_pool(name="rp", bufs=2))
            opool = ctx.enter_context(tc.tile_pool(name="orow", bufs=4))
            # PSUM: tag X (4 banks) = proj q/k accumulators | scores | out
            # tiles; tag Y (2 banks) = proj v accumulators | o-accumulators;
            # tag Z (2 banks) = v-transpose blocks | softmax-sum accumulators.
            psum = ctx.enter_context(tc.tile_pool(name="ps", bufs=2, space="PSUM"))

            # ---- persistent SBUF ----
            wq_sb = consts.tile([128, NJT, DL], sdt, name="wq_sb")
            wk_sb = consts.tile([128, NJT, DL], sdt, name="wk_sb")
            wv_sb = consts.tile([128, NJT, DL], sdt, name="wv_sb")
            wo_sb = consts.tile([128, HL, S], sdt, name="wo_sb")
            eg_sb = consts.tile([128, HL, eg_w], sdt, name="eg_sb")
            # [128, 128] all-ones stationary: the softmax-sum matmul then
            # produces Sum broadcast across all 128 partitions at no extra
            # PE cost (cycles scale with the moving width, not stationary m).
            ones_sb = consts.tile([128, 128], sdt, name="ones_sb")
            nc.vector.memset(ones_sb[:], 1.0)
            ident = consts.tile([128, 128], sdt, name="ident")
            make_identity(nc, ident[:])

            qT = persist.tile([128, HL, S], sdt)   # [d, h, i]
            kT = persist.tile([128, HL, S], sdt)   # [d, h, j]
            v_sb = persist.tile([128, HL, NJT, HD], sdt)  # [j, h, jt, d]
            o_sb = [
                persist.tile([128, HL, IB], sdt, name=f"o_sb{i}")
                for i in range(NIB)
            ]  # [d, h, i-block]

            for _rep in range(repeat):
                # ---- phase 1: projections (q^T, k^T, v^T), v transpose ----
                for ib in range(NIB):
                    isl = bass.ts(ib, IB)
                    xqs = []
                    for qd in range(NQ):
                        xq = xpool.tile([128, 4, IB], sdt, name="xq")
                        if _rep == 0 and ib == 0 and qd == 0:
                            # halve the very first transfers so the first
                            # matmul's operands land as early as possible
                            for hf in range(2):
                                cs = slice(hf * 2, hf * 2 + 2)
                                rs = slice(hf * 256, hf * 256 + 256)
                                # order: wq (first ldweights), x (first
                                # matmul), wk — minimizes the first stall
                                nc.sync.dma_start(
                                    out=wq_sb[:, cs, :],
                                    in_=wq_d[rs, :].rearrange(
                                        "(c p) d -> p c d", p=128
                                    ),
                                )
                                nc.sync.dma_start(
                                    out=xq[:, cs, :],
                                    in_=xT_d[rs, isl].rearrange(
                                        "(c p) i -> p c i", p=128
                                    ),
                                )
                                nc.sync.dma_start(
                                    out=wk_sb[:, cs, :],
                                    in_=wk_d[rs, :].rearrange(
                                        "(c p) d -> p c d", p=128
                                    ),
                                )
                            xqs.append(xq)
                            continue
                        nc.sync.dma_start(
                            out=xq[:],
                            in_=xT_d[qd * 512 : (qd + 1) * 512, isl].rearrange(
                                "(c p) i -> p c i", p=128
                            ),
                        )
                        xqs.append(xq)
                        if _rep == 0 and ib == 0:
                            # interleave weight quads with the first x quads
                            # so the first matmuls start ~3us in
                            for w_d, w_sb in ((wq_d, wq_sb), (wk_d, wk_sb)):
                                nc.sync.dma_start(
                                    out=w_sb[:, qd * 4 : (qd + 1) * 4, :],
                                    in_=w_d[
                                        qd * 512 : (qd + 1) * 512, :
                                    ].rearrange("(c p) d -> p c d", p=128),
                                )
                    if _rep == 0 and ib == 0:
                        for qd in range(NQ):
                            nc.sync.dma_start(
                                out=wv_sb[:, qd * 4 : (qd + 1) * 4, :],
                                in_=wv_d[
                                    qd * 512 : (qd + 1) * 512, :
                                ].rearrange("(c p) d -> p c d", p=128),
                            )
                    # pass A: q and k (4 PSUM banks, tag X)
                    psA = {}
                    for proj in range(2):
                        for dt_i in range(HL):
                            psA[(proj, dt_i)] = psum.tile(
                                [128, IB], f32, tag="X", bufs=4,
                                name=f"psA{proj}{dt_i}",
                            )
                    for ct in range(NJT):
                        xs = xqs[ct // 4][:, ct % 4, :]
                        for proj, w_sb in ((0, wq_sb), (1, wk_sb)):
                            for dt_i in range(HL):
                                nc.tensor.matmul(
                                    psA[(proj, dt_i)][:],
                                    lhsT=w_sb[:, ct, dt_i * HD : (dt_i + 1) * HD],
                                    rhs=xs,
                                    start=(ct == 0),
                                    stop=(ct == NJT - 1),
                                )
                    for dt_i in range(HL):
                        nc.scalar.activation(qT[:, dt_i, isl], psA[(0, dt_i)][:], Copy)
                        nc.vector.tensor_copy(kT[:, dt_i, isl], psA[(1, dt_i)][:])
                    # pass B: v (2 PSUM banks, tag Y)
                    psB = [
                        psum.tile([128, IB], f32, tag="Y", bufs=2, name=f"psB{d}")
                        for d in range(HL)
                    ]
                    for ct in range(NJT):
                        xs = xqs[ct // 4][:, ct % 4, :]
                        for dt_i in range(HL):
                            nc.tensor.matmul(
                                psB[dt_i][:],
                                lhsT=wv_sb[:, ct, dt_i * HD : (dt_i + 1) * HD],
                                rhs=xs,
                                start=(ct == 0),
                                stop=(ct == NJT - 1),
                            )
                    for dt_i in range(HL):
                        vt = vtpool.tile([128, IB], sdt, name="vt")
                        if dt_i == 0:
                            nc.vector.tensor_copy(vt[:], psB[dt_i][:])
                        else:
                            nc.scalar.activation(vt[:], psB[dt_i][:], Copy)
                        # transpose v^T [d, j] -> v [j, d] in 128-blocks
                        for s4 in range(IB // 128):
                            jt = (ib * IB) // 128 + s4
                            tp = psum.tile(
                                [128, 128], sdt, tag="Z", bufs=2, name="tp"
                            )
                            nc.tensor.transpose(
                                tp[:], vt[:, s4 * 128 : (s4 + 1) * 128], ident[:]
                            )
                            if s4 % 2 == 0:
                                nc.vector.tensor_copy(v_sb[:, dt_i, jt, :], tp[:])
                            else:
                                nc.scalar.activation(
                                    v_sb[:, dt_i, jt, :], tp[:], Copy
                                )

                if _rep == 0:
                    nc.sync.dma_start(
                        out=eg_sb[:], in_=eg_d.ap().rearrange("h p u -> p h u")
                    )
                    for h_i in range(HL):
                        nc.sync.dma_start(
                            out=wo_sb[:, h_i, :],
                            in_=wo_d[h_i * 128 : (h_i + 1) * 128, :],
                        )

                # ---- phase 2: attention, one global pipeline over blocks ----
                # Each tile computes only its prior-support columns [off, off+w)
                # of the i-block; the first tile of each accumulation group is
                # forced full-width so start=True zeroes the whole PSUM region.
                tasks = []
                for ib in range(NIB):
                    for h in range(HL):
                        jts = kept[h][ib]
                        for idx, (jt, off, w) in enumerate(jts):
                            if idx == 0:
                                off, w = 0, IB
                            tasks.append(
                                (ib, h, jt, off, w,
                                 idx == 0, idx == len(jts) - 1)
                            )

                block_acc = {}
                pend = deque()

                def emit_pv(t, pb):
                    ib, h, jt, off, w, first, last = t
                    if first:
                        block_acc[(ib, h)] = (
                            psum.tile([128, IB], f32, tag="Y", bufs=2, name="oacc"),
                            psum.tile([128, IB], f32, tag="Z", bufs=2, name="sacc"),
                        )
                    oacc, sacc = block_acc[(ib, h)]
                    nc.tensor.matmul(
                        oacc[:, off : off + w], lhsT=v_sb[:, h, jt, :],
                        rhs=pb[:, :w], start=first, stop=last,
                    )
                    nc.tensor.matmul(
                        sacc[:, off : off + w], lhsT=ones_sb[:],
                        rhs=pb[:, :w], start=first, stop=last,
                    )
                    if last:
                        rbc = rpool.tile([128, IB], f32, name="rbc")
                        nc.vector.reciprocal(rbc[:], sacc[:])
                        nc.vector.tensor_mul(o_sb[ib][:, h, :], oacc[:], rbc[:])

                for ti, t in enumerate(tasks):
                    ib, h, jt, off, w, first, last = t
                    sc = psum.tile([128, IB], f32, tag="X", bufs=4, name="sc")
                    nc.tensor.matmul(
                        sc[:, :w],
                        lhsT=kT[:, h, jt * 128 : (jt + 1) * 128],
                        rhs=qT[:, h, ib * IB + off : ib * IB + off + w],
                        start=True,
                        stop=True,
                    )
                    pb0 = ppool.tile([128, IB], sdt, tag="pb0", bufs=4, name="pb0")
                    nc.scalar.activation(pb0[:, :w], sc[:, :w], Exp)
                    pb = ppool.tile([128, IB], sdt, tag="pb", bufs=LAG + 2, name="pb")
                    base = ib * IB - jt * 128 + (S - 1) - eg_off + off
                    # eg multiply is SBUF->SBUF: give every 3rd one to Pool
                    (nc.gpsimd if ti % 3 == 2 else nc.vector).tensor_mul(
                        pb[:, :w], pb0[:, :w], eg_sb[:, h, base : base + w]
                    )
                    pend.append((t, pb))
                    if len(pend) > LAG:
                        emit_pv(*pend.popleft())
                while pend:
                    emit_pv(*pend.popleft())

                # ---- phase 3: out^T = wo^T @ (o^T/sum); host sums cores ----
                for et in range(NJT):
                    orow = opool.tile([128, S], sdt, name="orow")
                    for ib in range(NIB):
                        po = psum.tile([128, IB], f32, tag="X", bufs=4, name="po")
                        for h in range(HL):
                            nc.tensor.matmul(
                                po[:],
                                lhsT=wo_sb[:, h, et * 128 : (et + 1) * 128],
                                rhs=o_sb[ib][:, h, :],
                                start=(h == 0),
                                stop=(h == HL - 1),
                            )
                        osl = bass.ts(ib, IB)
                        if (et * NIB + ib) % 2 == 0:
                            nc.scalar.activation(orow[:, osl], po[:], Copy)
                        else:
                            nc.vector.tensor_copy(orow[:, osl], po[:])
                    nc.sync.dma_start(
                        out=outT_d[et * 128 : (et + 1) * 128, :], in_=orow[:]
                    )

    nc.compile()
    _PROGRAM_CACHE[key] = nc
    return nc


# ------------------------------------------------------------------- kernel
def prepare(x, mask, wq, wk, wv, wo, shape, scale, loc, start_pos):
    """Host prep: build/cache program and per-core input maps."""
    mm_name = MM_DTYPE
    np_store = ml_dtypes.bfloat16 if mm_name == "bf16" else np.float32

    x32 = np.asarray(x, np.float32).reshape(S, DIM)
    m2 = np.asarray(mask, np.float32).reshape(S, S)
    wq32 = np.asarray(wq, np.float32)
    wk32 = np.asarray(wk, np.float32)
    wv32 = np.asarray(wv, np.float32)
    wo32 = np.asarray(wo, np.float32)
    shape = np.asarray(shape, np.float32)
    scale = np.asarray(scale, np.float32)
    loc = np.asarray(loc, np.float32)
    sp = int(start_pos)

    phi = _toeplitz_profile(m2)
    if phi is None:
        raise ValueError("non-Toeplitz mask: use _numpy_fallback")

    # Per-head kept tiles from the prior window; sort heads narrow->wide and
    # give slot 0 the 8 narrowest (the SPMD program computes the per-slot
    # union, so grouping similar windows minimizes wasted tiles).
    head_kept = []
    profiles = []
    for g in range(H):
        eg1 = _eg_profile(g, shape, scale, loc, sp, phi)
        profiles.append(eg1)
        head_kept.append(_head_kept_tiles(eg1))
    counts = [sum(len(r) for r in k) for k in head_kept]
    perm = list(np.argsort(np.asarray(counts), kind="stable"))
    kept_slots = []
    full_w = narrow_w = 0
    for s in range(HL):
        heads = perm[s * N_CORES : (s + 1) * N_CORES]
        eg_slot = np.maximum.reduce([profiles[g] for g in heads]).astype(
            np.float32
        )
        slot_rows = []
        for ib in range(NIB):
            jts = sorted(set().union(*[set(head_kept[g][ib]) for g in heads]))
            row = []
            for jt in jts:
                off, w = _tile_support(eg_slot, ib, jt)
                row.append((jt, off, w))
                full_w += IB
                narrow_w += w
            slot_rows.append(row)
        kept_slots.append(slot_rows)
    LAST_RUN_INFO["narrow_frac"] = narrow_w / max(full_w, 1)
    kept_key = tuple(
        tuple(tuple(row) for row in slot) for slot in kept_slots
    )
    eg_off, eg_w = _eg_geometry(kept_slots)

    LAST_RUN_INFO["build_args"] = (mm_name, kept_key, eg_off, eg_w)
    LAST_RUN_INFO["perm"] = perm
    nc = _build_program(mm_name, kept_key, eg_off, eg_w)

    xT = np.ascontiguousarray(x32.T).astype(np_store)
    inv_s = np.float32(1.0 / math.sqrt(HD))

    in_maps = []
    for c in range(N_CORES):
        heads = [perm[c], perm[N_CORES + c]]
        cols = np.concatenate(
            [np.arange(g * HD, (g + 1) * HD) for g in heads]
        )
        im = {
            "xT": xT,
            "wq": np.ascontiguousarray(wq32[:, cols] * inv_s).astype(np_store),
            "wk": np.ascontiguousarray(wk32[:, cols]).astype(np_store),
            "wv": np.ascontiguousarray(wv32[:, cols]).astype(np_store),
            "wo": np.ascontiguousarray(wo32[cols, :]).astype(np_store),
            "eg": np.stack(
                [
                    _eg_table(g, shape, scale, loc, sp, phi, eg_off, eg_w)
                    for g in heads
                ]
            ).astype(np_store),
        }
        in_maps.append(im)
    return nc, in_maps


def _numpy_fallback(x, mask, wq, wk, wv, wo, shape, scale, loc, start_pos):
    x2 = np.asarray(x, np.float32)[0]
    m = np.asarray(mask, np.float32)[0, 0]
    wq = np.asarray(wq, np.float32)
    wk = np.asarray(wk, np.float32)
    wv = np.asarray(wv, np.float32)
    wo = np.asarray(wo, np.float32)
    shape = np.asarray(shape, np.float32)
    scale = np.asarray(scale, np.float32)
    loc = np.asarray(loc, np.float32)
    sp = int(start_pos)
    q = (x2 @ wq).reshape(S, H, HD)
    k = (x2 @ wk).reshape(S, H, HD)
    v = (x2 @ wv).reshape(S, H, HD)
    out = np.zeros((S, H * HD), np.float32)
    qpos = np.arange(S, dtype=np.float32) + sp
    kpos = np.arange(S, dtype=np.float32)
    dist = kpos[None, :] - qpos[:, None]
    for h in range(H):
        s = (q[:, h] @ k[:, h].T) / np.float32(math.sqrt(HD))
        lo = loc[0, h, 0, 0]
        loc_t = np.exp(lo) - np.exp(-lo)
        z = (dist - loc_t) * np.exp(scale[0, h, 0, 0])
        s = s - (np.abs(z) + np.float32(EPS)) ** shape[0, h, 0, 0]
        s = s + m
        e = np.exp(s - s.max(axis=-1, keepdims=True))
        p = e / e.sum(axis=-1, keepdims=True)
        out[:, h * HD : (h + 1) * HD] = p @ v[:, h]
    return (out @ wo)[None].astype(np.float32)


def _reduce(results):
    acc = results[0]["outT"].astype(np.float32)
    for c in range(1, N_CORES):
        acc = acc + results[c]["outT"].astype(np.float32)
    return np.ascontiguousarray(acc.T)[None].astype(np.float32)


_RUNNER_CACHE = {}


def _get_runner(nc):
    """Build (once) a reusable jitted 8-core runner for the program `nc`.
    Mirrors bass2jax.run_bass_via_pjrt's multi-core path without output
    donation (outT is fully written by the kernel) so it can be re-invoked."""
    if id(nc) in _RUNNER_CACHE:
        return _RUNNER_CACHE[id(nc)]

    import jax
    from jax.sharding import Mesh, NamedSharding, PartitionSpec

    from jax.experimental.shard_map import shard_map
    from concourse import mybir
    from concourse.bass2jax import (
        _bass_exec_p,
        install_neuronx_cc_hook,
        partition_id_tensor,
    )

    install_neuronx_cc_hook()
    partition_name = nc.partition_id_tensor.name if nc.partition_id_tensor else None

    in_names, out_names, out_avals = [], [], []
    for alloc in nc.m.functions[0].allocations:
        if not isinstance(alloc, mybir.MemoryLocationSet):
            continue
        name = alloc.memorylocations[0].name
        if alloc.kind == "ExternalInput":
            if name != partition_name:
                in_names.append(name)
        elif alloc.kind == "ExternalOutput":
            out_names.append(name)
            out_avals.append(
                jax.core.ShapedArray(
                    tuple(alloc.tensor_shape), mybir.dt.np(alloc.dtype)
                )
            )
    n_params = len(in_names)
    all_names = in_names + out_names
    if partition_name is not None:
        all_names = all_names + [partition_name]

    def _body(*args):
        operands = list(args)
        if partition_name is not None:
            operands.append(partition_id_tensor())
        return tuple(
            _bass_exec_p.bind(
                *operands,
                out_avals=tuple(out_avals),
                in_names=tuple(all_names),
                out_names=tuple(out_names),
                lowering_input_output_aliases=(),
                sim_require_finite=True,
                sim_require_nnan=True,
                nc=nc,
            )
        )

    devices = jax.devices()[:N_CORES]
    mesh = Mesh(np.asarray(devices), ("core",))
    sharded = jax.jit(
        shard_map(
            _body,
            mesh=mesh,
            in_specs=(PartitionSpec("core"),) * (n_params + len(out_names)),
            out_specs=(PartitionSpec("core"),) * len(out_names),
            check_rep=False,
        ),
        keep_unused=True,
    )
    sh = NamedSharding(mesh, PartitionSpec("core"))

    def run(in_maps):
        concat_in = [
            np.concatenate(
                [np.asarray(in_maps[c][nm]) for c in range(N_CORES)], axis=0
            )
            for nm in in_names
        ]
        concat_zero = [
            np.zeros((N_CORES * av.shape[0], *av.shape[1:]), av.dtype)
            for av in out_avals
        ]
        dev_args = [jax.device_put(a, sh) for a in concat_in + concat_zero]
        out_arrs = sharded(*dev_args)
        return [
            {
                nm: np.asarray(out_arrs[i]).reshape(
                    N_CORES, *out_avals[i].shape
                )[c]
                for i, nm in enumerate(out_names)
            }
            for c in range(N_CORES)
        ]

    _RUNNER_CACHE[id(nc)] = run
    return run


def kernel(x, mask, wq, wk, wv, wo, shape, scale, loc, start_pos):
    m2 = np.asarray(mask, np.float32).reshape(S, S)
    if _toeplitz_profile(m2) is None:
        return _numpy_fallback(
            x, mask, wq, wk, wv, wo, shape, scale, loc, start_pos
        )
    nc, in_maps = prepare(x, mask, wq, wk, wv, wo, shape, scale, loc, start_pos)

    if os.environ.get("KBA_SIM", "0") == "1":
        from concourse import bass_interp

        n_sim = int(os.environ.get("KBA_SIM_CORES", str(N_CORES)))
        sim = bass_interp.MultiCoreSim(nc, n_sim)
        for c in range(n_sim):
            for k, v in in_maps[c].items():
                sim.cores[c].tensor(k)[:] = v
        sim.simulate()
        results = [
            {"outT": np.array(sim.cores[c].tensor("outT"), np.float32)}
            for c in range(n_sim)
        ] + [
            {"outT": np.zeros((S, S), np.float32)} for _ in range(N_CORES - n_sim)
        ]
        LAST_RUN_INFO["exec_time_ns"] = None
    else:
        results = _get_runner(nc)(in_maps)
        LAST_RUN_INFO["exec_time_ns"] = None

    LAST_RUN_INFO["results"] = results
    return _reduce(results)


# revision 24
# speedup vs baseline: 1.0499x; 1.0499x over previous
"""BayesianAttention (ALiBi-style power-law prior + causal mask) on 8 trn2 cores.

Self-contained: builds a Bass/Tile kernel, shards heads across 8 NeuronCores
(2 heads per core; wq/wk/wv column-sharded, wo row-sharded), runs via a
shard_map'd bass program, and reduces the partial outputs on host.

v2 layout (all-bf16 operands, batched DMA, software-pipelined attention):
  host sends x^T [c, i] bf16; device computes q^T/k^T/v^T = W^T x^T in two
  PSUM passes (q,k then v), transposes v, s^T[j,i] = k^T_j . q^T_i,
  probs = exp(s^T) * EG  (EG = exp(prior + mask), a Toeplitz table indexed
  by j - i, precomputed on host, 0 where masked), o^T[d,i] = v^T probs with
  softmax sum via an all-ones stationary matmul, out^T[e,i] = wo^T (o^T/sum).
  Host returns sum_c(out^T_c)^T.

  All DMAs are batched (few large transfers) and issued on the SP queue;
  the attention inner loop is one global software pipeline across (ib, h)
  blocks so the PE never drains between blocks; phase-3 output rows are
  accumulated in SBUF and written once per 128-row stripe.
"""

import math
import os
from collections import deque

import ml_dtypes
import numpy as np

S = 2048          # sequence length
DIM = 2048        # model dim
H = 16            # heads
HD = 128          # head dim
N_CORES = 8
HL = H // N_CORES  # heads per core (2)
DL = HL * HD       # local projected dim (256)
IB = 512           # i-block (query block, moving free dim)
NIB = S // IB
NJT = S // 128     # key tiles of 128
NQ = 4             # x row-quads (512 rows each)
EPS = 1e-5
MASKED_THRESH = -1e8   # additive mask values below this mean "fully masked"

MM_DTYPE = os.environ.get("KBA_DTYPE", "bf16")  # "bf16" | "f32"
LAG = int(os.environ.get("KBA_LAG", "4"))  # scores->o-matmul emission lag
# every k-th eg-multiply runs on Pool instead of DVE (0 = all on DVE; Pool's
# low-efficiency multiply adds latency to the probs chain, so DVE-only wins)
MULPOOL = int(os.environ.get("KBA_MULPOOL", "0"))

LAST_RUN_INFO = {}


# ---------------------------------------------------------------- tile patch
def _apply_tile_patch():
    """walrus CoreV3 codegen tolerates only one sync-wait on an InstDrain;
    the tile-exit drain waits on the whole global clock. Spread the waits
    across extra SP nops."""
    import concourse.tile as tile
    from concourse import mybir
    from concourse.vector_clock import ScopedClock

    if getattr(tile.TileContext, "_kba_patched", False):
        return

    def _drain_and_barrier(self, tick_clock, wait_clock):
        nc = self.nc
        drain_inst = nc.sync.drain()
        wait_clock.add_sem_waits(
            drain_inst.ins, ScopedClock({None: tick_clock.global_clock})
        )
        si = drain_inst.ins.sync_info
        waits = list(si.on_wait or [])
        if len(waits) > 1:
            si.on_wait = waits[:1]
            for i in range(1, len(waits)):
                nop = nc.sync.nop(nofuse=True)
                nop.ins.sync_info = mybir.SyncInfo(
                    on_wait=waits[i : i + 1], on_update=[]
                )
        nc.all_engine_barrier()
        assert self.sems is not None
        popped = nc._tile_sem_poison_stack.pop()
        assert popped is self._sem_poison
        nc.clear_and_free_semaphores(list(self.sems.allocated().values()))
        nc.all_engine_barrier()

    tile.TileContext._drain_and_barrier = _drain_and_barrier
    tile.TileContext._kba_patched = True

    try:
        import concourse.tile_utils as tile_utils

        tile_utils.max_sbuf_usage = 208 * 1024
    except Exception:
        pass


# ------------------------------------------------------------- host helpers
def _toeplitz_profile(m2):
    """If mask[i, j] == phi(j - i) for all i,j, return phi (length 2S-1,
    index t + S - 1), else None."""
    phi = np.empty(2 * S - 1, dtype=np.float32)
    phi[S - 1 :] = m2[0, :]
    phi[: S - 1] = m2[1:, 0][::-1]
    idx = (np.arange(S)[None, :] - np.arange(S)[:, None]) + (S - 1)
    if np.array_equal(phi[idx], m2):
        return phi
    return None


# tiles/columns whose max prior weight is below this contribute nothing:
# host dry-run shows even element-granularity dropping at 1e-4 leaves the
# output bit-identical (errors move only at 1e-3)
EG_TAU = 1e-4


def _eg_profile(head, shape, scale, loc, start_pos, phi):
    """1-D prior weight profile eg1[d + (S-1)] = exp(prior(d) + phi(-d)) for
    d = i - j in [-(S-1), S-1]. The 2-D EG table is eg1 evaluated per (p,u);
    a tile's max EG is the max of eg1 over the tile's contiguous d-range."""
    d = np.arange(-(S - 1), S, dtype=np.int64)
    dist = (-d - start_pos).astype(np.float32)
    sh = np.float32(shape[0, head, 0, 0])
    sc = np.float32(scale[0, head, 0, 0])
    lo = np.float32(loc[0, head, 0, 0])
    loc_t = np.float32(np.exp(lo) - np.exp(-lo))
    z = (dist - loc_t) * np.exp(sc, dtype=np.float32)
    g = -np.power(np.abs(z) + np.float32(EPS), sh, dtype=np.float32)
    g = g + phi[np.clip(-d + (S - 1), 0, 2 * S - 2)]
    return np.exp(g)


def _head_kept_tiles(eg1):
    """kept[ib] = j-tiles where the head's prior x mask weight is non-negligible
    somewhere in the [IB x 128] block."""
    kept = []
    for ib in range(NIB):
        row = []
        for jt in range(NJT):
            dlo = ib * IB - jt * 128 - 127
            dhi = ib * IB + IB - 1 - jt * 128
            lo = max(dlo + (S - 1), 0)
            hi = min(dhi + (S - 1), 2 * S - 2)
            if lo <= hi and eg1[lo : hi + 1].max() >= EG_TAU:
                row.append(jt)
        kept.append(row)
    return kept


def _tile_support(eg_slot, ib, jt):
    """(off, w): the i-column range of block ib where tile jt has any
    non-negligible prior x mask weight. eg_slot is the per-slot max of the
    heads' 1-D profiles; column i_local covers d = (ib*IB + i_local) - j for
    j in the tile, a 128-wide window of eg_slot."""
    pad = np.zeros(2 * S - 1 + 256, np.float32)
    pad[128 : 128 + 2 * S - 1] = eg_slot
    # max over the 128-wide window ending at d = ib*IB + i - jt*128
    idx = (ib * IB + np.arange(IB) - jt * 128) + (S - 1) + 128
    win = np.lib.stride_tricks.sliding_window_view(pad, 128)
    colmax = win[idx - 127].max(axis=1)
    on = np.nonzero(colmax >= EG_TAU)[0]
    assert on.size, "kept tile with empty support"
    off = int(on[0]) & ~3
    end = min(IB, (int(on[-1]) + 1 + 3) & ~3)
    return off, end - off


def _eg_geometry(kept_slots):
    """The EG table covers u = base..base+IB for every kept (ib, jt), where
    base = ib*IB - jt*128 + (S-1). Returns (offset, width)."""
    bases = [
        ib * IB - jt * 128 + (S - 1)
        for kept in kept_slots
        for ib in range(NIB)
        for jt, _, _ in kept[ib]
    ]
    off = min(bases)
    width = max(bases) + IB - off
    return off, width


def _eg_table(head, shape, scale, loc, start_pos, phi, eg_off, eg_w):
    """[128, eg_w] float32: EG[p, u'] = exp(prior(d) + phi(-d)), 0 where
    masked/out of range, with d = i - j = (u' + eg_off) - p - (S - 1)."""
    p = np.arange(128, dtype=np.int64)[:, None]
    u = eg_off + np.arange(eg_w, dtype=np.int64)[None, :]
    d = u - p - (S - 1)          # i - j
    dist = (-d - start_pos).astype(np.float32)  # k_pos - q_pos
    sh = np.float32(shape[0, head, 0, 0])
    sc = np.float32(scale[0, head, 0, 0])
    lo = np.float32(loc[0, head, 0, 0])
    loc_t = np.float32(np.exp(lo) - np.exp(-lo))
    z = (dist - loc_t) * np.exp(sc, dtype=np.float32)
    g = -np.power(np.abs(z) + np.float32(EPS), sh, dtype=np.float32)
    t = np.clip(-d + (S - 1), 0, 2 * S - 2)
    g = g + phi[t]
    g[(-d < -(S - 1)) | (-d > (S - 1))] = -np.inf  # out of range: never read
    return np.ascontiguousarray(np.exp(g).astype(np.float32))


# ------------------------------------------------------------ program build
_PROGRAM_CACHE = {}


def _build_program(mm_name, kept_key, eg_off, eg_w, repeat=1):
    key = (mm_name, kept_key, eg_off, eg_w, repeat)
    if key in _PROGRAM_CACHE:
        return _PROGRAM_CACHE[key]

    import concourse.bass as bass
    import concourse.tile as tile
    from concourse import bacc, mybir
    from concourse.masks import make_identity

    _apply_tile_patch()

    f32 = mybir.dt.float32
    sdt = mybir.dt.bfloat16 if mm_name == "bf16" else f32

    # kept_key[h][ib] = j-tiles for local head slot h (slot 0 carries the
    # narrow-window heads, slot 1 the wide ones; identical across cores)
    kept = [[list(row) for row in slot] for slot in kept_key]

    nc = bacc.Bacc(
        "TRN2", target_bir_lowering=False, debug=False, num_devices=N_CORES
    )
    xT_d = nc.dram_tensor("xT", [S, S], sdt, kind="ExternalInput")
    wq_d = nc.dram_tensor("wq", [S, DL], sdt, kind="ExternalInput")
    wk_d = nc.dram_tensor("wk", [S, DL], sdt, kind="ExternalInput")
    wv_d = nc.dram_tensor("wv", [S, DL], sdt, kind="ExternalInput")
    wo_d = nc.dram_tensor("wo", [DL, S], sdt, kind="ExternalInput")
    eg_d = nc.dram_tensor("eg", [HL, 128, eg_w], sdt, kind="ExternalInput")
    outT_d = nc.dram_tensor("outT", [S, S], sdt, kind="ExternalOutput")

    Exp = mybir.ActivationFunctionType.Exp
    Copy = mybir.ActivationFunctionType.Copy

    with tile.TileContext(nc) as tc:
        import contextlib

        with contextlib.ExitStack() as ctx:
            consts = ctx.enter_context(tc.tile_pool(name="consts", bufs=1))
            persist = ctx.enter_context(tc.tile_pool(name="persist", bufs=1))
            xpool = ctx.enter_context(tc.tile_pool(name="xp", bufs=8))
            ppool = ctx.enter_context(tc.tile_pool(name="probs", bufs=4))
            vtpool = ctx.enter_context(tc.tile_pool(name="vt", bufs=2))
            rpool = ctx.enter_context(tc.tile_pool(name="rp", bufs=2))
            opool = ctx.enter_context(tc.tile_pool(name="orow", bufs=4))
            # PSUM: tag X (4 banks) = proj q/k accumulators | scores | out
            # tiles; tag Y (2 banks) = proj v accumulators | o-accumulators;
            # tag Z (2 banks) = v-transpose blocks | softmax-sum accumulators.
            psum = ctx.enter_context(tc.tile_pool(name="ps", bufs=2, space="PSUM"))

            # ---- persistent SBUF ----
            wq_sb = consts.tile([128, NJT, DL], sdt, name="wq_sb")
            wk_sb = consts.tile([128, NJT, DL], sdt, name="wk_sb")
            wv_sb = consts.tile([128, NJT, DL], sdt, name="wv_sb")
            wo_sb = consts.tile([128, HL, S], sdt, name="wo_sb")
            eg_sb = consts.tile([128, HL, eg_w], sdt, name="eg_sb")
            # [128, 128] all-ones stationary: the softmax-sum matmul then
            # produces Sum broadcast across all 128 partitions at no extra
            # PE cost (cycles scale with the moving width, not stationary m).
            ones_sb = consts.tile([128, 128], sdt, name="ones_sb")
            nc.vector.memset(ones_sb[:], 1.0)
            ident = consts.tile([128, 128], sdt, name="ident")
            make_identity(nc, ident[:])

            qT = persist.tile([128, HL, S], sdt)   # [d, h, i]
            kT = persist.tile([128, HL, S], sdt)   # [d, h, j]
            v_sb = persist.tile([128, HL, NJT, HD], sdt)  # [j, h, jt, d]
            o_sb = [
                persist.tile([128, HL, IB], sdt, name=f"o_sb{i}")
                for i in range(NIB)
            ]  # [d, h, i-block]

            for _rep in range(repeat):
                # ---- phase 1: projections (q^T, k^T, v^T), v transpose ----
                for ib in range(NIB):
                    isl = bass.ts(ib, IB)
                    xqs = []
                    for qd in range(NQ):
                        xq = xpool.tile([128, 4, IB], sdt, name="xq")
                        if _rep == 0 and ib == 0 and qd == 0:
                            # halve the very first transfers so the first
                            # matmul's operands land as early as possible
                            for hf in range(2):
                                cs = slice(hf * 2, hf * 2 + 2)
                                rs = slice(hf * 256, hf * 256 + 256)
                                # order: wq (first ldweights), x (first
                                # matmul), wk — minimizes the first stall
                                nc.sync.dma_start(
                                    out=wq_sb[:, cs, :],
                                    in_=wq_d[rs, :].rearrange(
                                        "(c p) d -> p c d", p=128
                                    ),
                                )
                                nc.sync.dma_start(
                                    out=xq[:, cs, :],
                                    in_=xT_d[rs, isl].rearrange(
                                        "(c p) i -> p c i", p=128
                                    ),
                                )
                                nc.sync.dma_start(
                                    out=wk_sb[:, cs, :],
                                    in_=wk_d[rs, :].rearrange(
                                        "(c p) d -> p c d", p=128
                                    ),
                                )
                            xqs.append(xq)
                            continue
                        nc.sync.dma_start(
                            out=xq[:],
                            in_=xT_d[qd * 512 : (qd + 1) * 512, isl].rearrange(
                                "(c p) i -> p c i", p=128
                            ),
                        )
                        xqs.append(xq)
                        if _rep == 0 and ib == 0:
                            # interleave weight quads with the first x quads
                            # so the first matmuls start ~3us in
                            for w_d, w_sb in ((wq_d, wq_sb), (wk_d, wk_sb)):
                                nc.sync.dma_start(
                                    out=w_sb[:, qd * 4 : (qd + 1) * 4, :],
                                    in_=w_d[
                                        qd * 512 : (qd + 1) * 512, :
                                    ].rearrange("(c p) d -> p c d", p=128),
                                )
                    if _rep == 0 and ib == 0:
                        for qd in range(NQ):
                            nc.sync.dma_start(
                                out=wv_sb[:, qd * 4 : (qd + 1) * 4, :],
                                in_=wv_d[
                                    qd * 512 : (qd + 1) * 512, :
                                ].rearrange("(c p) d -> p c d", p=128),
                            )
                    # pass A: q and k (4 PSUM banks, tag X)
                    psA = {}
                    for proj in range(2):
                        for dt_i in range(HL):
                            psA[(proj, dt_i)] = psum.tile(
                                [128, IB], f32, tag="X", bufs=4,
                                name=f"psA{proj}{dt_i}",
                            )
                    for ct in range(NJT):
                        xs = xqs[ct // 4][:, ct % 4, :]
                        for proj, w_sb in ((0, wq_sb), (1, wk_sb)):
                            for dt_i in range(HL):
                                nc.tensor.matmul(
                                    psA[(proj, dt_i)][:],
                                    lhsT=w_sb[:, ct, dt_i * HD : (dt_i + 1) * HD],
                                    rhs=xs,
                                    start=(ct == 0),
                                    stop=(ct == NJT - 1),
                                )
                    for dt_i in range(HL):
                        nc.scalar.activation(qT[:, dt_i, isl], psA[(0, dt_i)][:], Copy)
                        nc.vector.tensor_copy(kT[:, dt_i, isl], psA[(1, dt_i)][:])
                    # pass B: v (2 PSUM banks, tag Y)
                    psB = [
                        psum.tile([128, IB], f32, tag="Y", bufs=2, name=f"psB{d}")
                        for d in range(HL)
                    ]
                    for ct in range(NJT):
                        xs = xqs[ct // 4][:, ct % 4, :]
                        for dt_i in range(HL):
                            nc.tensor.matmul(
                                psB[dt_i][:],
                                lhsT=wv_sb[:, ct, dt_i * HD : (dt_i + 1) * HD],
                                rhs=xs,
                                start=(ct == 0),
                                stop=(ct == NJT - 1),
                            )
                    for dt_i in range(HL):
                        vt = vtpool.tile([128, IB], sdt, name="vt")
                        if dt_i == 0:
                            nc.vector.tensor_copy(vt[:], psB[dt_i][:])
                        else:
                            nc.scalar.activation(vt[:], psB[dt_i][:], Copy)
                        # transpose v^T [d, j] -> v [j, d] in 128-blocks
                        for s4 in range(IB // 128):
                            jt = (ib * IB) // 128 + s4
                            tp = psum.tile(
                                [128, 128], sdt, tag="Z", bufs=2, name="tp"
                            )
                            nc.tensor.transpose(
                                tp[:], vt[:, s4 * 128 : (s4 + 1) * 128], ident[:]
                            )
                            if s4 % 2 == 0:
                                nc.vector.tensor_copy(v_sb[:, dt_i, jt, :], tp[:])
                            else:
                                nc.scalar.activation(
                                    v_sb[:, dt_i, jt, :], tp[:], Copy
                                )

                if _rep == 0:
                    nc.sync.dma_start(
                        out=eg_sb[:], in_=eg_d.ap().rearrange("h p u -> p h u")
                    )
                    for h_i in range(HL):
                        nc.sync.dma_start(
                            out=wo_sb[:, h_i, :],
                            in_=wo_d[h_i * 128 : (h_i + 1) * 128, :],
                        )

                # ---- phase 2: attention, one global pipeline over blocks ----
                # Each tile computes only its prior-support columns [off, off+w)
                # of the i-block; the first tile of each accumulation group is
                # forced full-width so start=True zeroes the whole PSUM region.
                tasks = []
                for ib in range(NIB):
                    for h in range(HL):
                        jts = kept[h][ib]
                        for idx, (jt, off, w) in enumerate(jts):
                            if idx == 0:
                                off, w = 0, IB
                            tasks.append(
                                (ib, h, jt, off, w,
                                 idx == 0, idx == len(jts) - 1)
                            )

                block_acc = {}
                pend = deque()

                def emit_pv(t, pb):
                    ib, h, jt, off, w, first, last = t
                    if first:
                        block_acc[(ib, h)] = (
                            psum.tile([128, IB], f32, tag="Y", bufs=2, name="oacc"),
                            psum.tile([128, IB], f32, tag="Z", bufs=2, name="sacc"),
                        )
                    oacc, sacc = block_acc[(ib, h)]
                    nc.tensor.matmul(
                        oacc[:, off : off + w], lhsT=v_sb[:, h, jt, :],
                        rhs=pb[:, :w], start=first, stop=last,
                    )
                    nc.tensor.matmul(
                        sacc[:, off : off + w], lhsT=ones_sb[:],
                        rhs=pb[:, :w], start=first, stop=last,
                    )
                    if last:
                        rbc = rpool.tile([128, IB], f32, name="rbc")
                        nc.vector.reciprocal(rbc[:], sacc[:])
                        nc.vector.tensor_mul(o_sb[ib][:, h, :], oacc[:], rbc[:])

                for ti, t in enumerate(tasks):
                    ib, h, jt, off, w, first, last = t
                    sc = psum.tile([128, IB], f32, tag="X", bufs=4, name="sc")
                    nc.tensor.matmul(
                        sc[:, :w],
                        lhsT=kT[:, h, jt * 128 : (jt + 1) * 128],
                        rhs=qT[:, h, ib * IB + off : ib * IB + off + w],
                        start=True,
                        stop=True,
                    )
                    pb0 = ppool.tile([128, IB], sdt, tag="pb0", bufs=4, name="pb0")
                    nc.scalar.activation(pb0[:, :w], sc[:, :w], Exp)
                    pb = ppool.tile([128, IB], sdt, tag="pb", bufs=LAG + 2, name="pb")
                    base = ib * IB - jt * 128 + (S - 1) - eg_off + off
                    # eg multiply is SBUF->SBUF: offload a share to Pool
                    on_pool = MULPOOL > 0 and ti % MULPOOL == MULPOOL - 1
                    (nc.gpsimd if on_pool else nc.vector).tensor_mul(
                        pb[:, :w], pb0[:, :w], eg_sb[:, h, base : base + w]
                    )
                    pend.append((t, pb))
                    if len(pend) > LAG:
                        emit_pv(*pend.popleft())
                while pend:
                    emit_pv(*pend.popleft())

                # ---- phase 3: out^T = wo^T @ (o^T/sum); host sums cores ----
                for et in range(NJT):
                    orow = opool.tile([128, S], sdt, name="orow")
                    for ib in range(NIB):
                        po = psum.tile([128, IB], f32, tag="X", bufs=4, name="po")
                        for h in range(HL):
                            nc.tensor.matmul(
                                po[:],
                                lhsT=wo_sb[:, h, et * 128 : (et + 1) * 128],
                                rhs=o_sb[ib][:, h, :],
                                start=(h == 0),
                                stop=(h == HL - 1),
                            )
                        osl = bass.ts(ib, IB)
                        if (et * NIB + ib) % 2 == 0:
                            nc.scalar.activation(orow[:, osl], po[:], Copy)
                        else:
                            nc.vector.tensor_copy(orow[:, osl], po[:])
                    nc.sync.dma_start(
                        out=outT_d[et * 128 : (et + 1) * 128, :], in_=orow[:]
                    )

    nc.compile()
    _PROGRAM_CACHE[key] = nc
    return nc


# ------------------------------------------------------------------- kernel
def prepare(x, mask, wq, wk, wv, wo, shape, scale, loc, start_pos):
    """Host prep: build/cache program and per-core input maps."""
    mm_name = MM_DTYPE
    np_store = ml_dtypes.bfloat16 if mm_name == "bf16" else np.float32

    x32 = np.asarray(x, np.float32).reshape(S, DIM)
    m2 = np.asarray(mask, np.float32).reshape(S, S)
    wq32 = np.asarray(wq, np.float32)
    wk32 = np.asarray(wk, np.float32)
    wv32 = np.asarray(wv, np.float32)
    wo32 = np.asarray(wo, np.float32)
    shape = np.asarray(shape, np.float32)
    scale = np.asarray(scale, np.float32)
    loc = np.asarray(loc, np.float32)
    sp = int(start_pos)

    phi = _toeplitz_profile(m2)
    if phi is None:
        raise ValueError("non-Toeplitz mask: use _numpy_fallback")

    # Per-head kept tiles from the prior window; sort heads narrow->wide and
    # give slot 0 the 8 narrowest (the SPMD program computes the per-slot
    # union, so grouping similar windows minimizes wasted tiles).
    head_kept = []
    profiles = []
    for g in range(H):
        eg1 = _eg_profile(g, shape, scale, loc, sp, phi)
        profiles.append(eg1)
        head_kept.append(_head_kept_tiles(eg1))
    counts = [sum(len(r) for r in k) for k in head_kept]
    perm = list(np.argsort(np.asarray(counts), kind="stable"))
    kept_slots = []
    full_w = narrow_w = 0
    for s in range(HL):
        heads = perm[s * N_CORES : (s + 1) * N_CORES]
        eg_slot = np.maximum.reduce([profiles[g] for g in heads]).astype(
            np.float32
        )
        slot_rows = []
        for ib in range(NIB):
            jts = sorted(set().union(*[set(head_kept[g][ib]) for g in heads]))
            row = []
            for jt in jts:
                off, w = _tile_support(eg_slot, ib, jt)
                row.append((jt, off, w))
                full_w += IB
                narrow_w += w
            slot_rows.append(row)
        kept_slots.append(slot_rows)
    LAST_RUN_INFO["narrow_frac"] = narrow_w / max(full_w, 1)
    kept_key = tuple(
        tuple(tuple(row) for row in slot) for slot in kept_slots
    )
    eg_off, eg_w = _eg_geometry(kept_slots)

    LAST_RUN_INFO["build_args"] = (mm_name, kept_key, eg_off, eg_w)
    LAST_RUN_INFO["perm"] = perm
    nc = _build_program(mm_name, kept_key, eg_off, eg_w)

    xT = np.ascontiguousarray(x32.T).astype(np_store)
    inv_s = np.float32(1.0 / math.sqrt(HD))

    in_maps = []
    for c in range(N_CORES):
        heads = [perm[c], perm[N_CORES + c]]
        cols = np.concatenate(
            [np.arange(g * HD, (g + 1) * HD) for g in heads]
        )
        im = {
            "xT": xT,
            "wq": np.ascontiguousarray(wq32[:, cols] * inv_s).astype(np_store),
            "wk": np.ascontiguousarray(wk32[:, cols]).astype(np_store),
            "wv": np.ascontiguousarray(wv32[:, cols]).astype(np_store),
            "wo": np.ascontiguousarray(wo32[cols, :]).astype(np_store),
            "eg": np.stack(
                [
                    _eg_table(g, shape, scale, loc, sp, phi, eg_off, eg_w)
                    for g in heads
                ]
            ).astype(np_store),
        }
        in_maps.append(im)
    return nc, in_maps


def _numpy_fallback(x, mask, wq, wk, wv, wo, shape, scale, loc, start_pos):
    x2 = np.asarray(x, np.float32)[0]
    m = np.asarray(mask, np.float32)[0, 0]
    wq = np.asarray(wq, np.float32)
    wk = np.asarray(wk, np.float32)
    wv = np.asarray(wv, np.float32)
    wo = np.asarray(wo, np.float32)
    shape = np.asarray(shape, np.float32)
    scale = np.asarray(scale, np.float32)
    loc = np.asarray(loc, np.float32)
    sp = int(start_pos)
    q = (x2 @ wq).reshape(S, H, HD)
    k = (x2 @ wk).reshape(S, H, HD)
    v = (x2 @ wv).reshape(S, H, HD)
    out = np.zeros((S, H * HD), np.float32)
    qpos = np.arange(S, dtype=np.float32) + sp
    kpos = np.arange(S, dtype=np.float32)
    dist = kpos[None, :] - qpos[:, None]
    for h in range(H):
        s = (q[:, h] @ k[:, h].T) / np.float32(math.sqrt(HD))
        lo = loc[0, h, 0, 0]
        loc_t = np.exp(lo) - np.exp(-lo)
        z = (dist - loc_t) * np.exp(scale[0, h, 0, 0])
        s = s - (np.abs(z) + np.float32(EPS)) ** shape[0, h, 0, 0]
        s = s + m
        e = np.exp(s - s.max(axis=-1, keepdims=True))
        p = e / e.sum(axis=-1, keepdims=True)
        out[:, h * HD : (h + 1) * HD] = p @ v[:, h]
    return (out @ wo)[None].astype(np.float32)


def _reduce(results):
    acc = results[0]["outT"].astype(np.float32)
    for c in range(1, N_CORES):
        acc = acc + results[c]["outT"].astype(np.float32)
    return np.ascontiguousarray(acc.T)[None].astype(np.float32)


_RUNNER_CACHE = {}


def _get_runner(nc):
    """Build (once) a reusable jitted 8-core runner for the program `nc`.
    Mirrors bass2jax.run_bass_via_pjrt's multi-core path without output
    donation (outT is fully written by the kernel) so it can be re-invoked."""
    if id(nc) in _RUNNER_CACHE:
        return _RUNNER_CACHE[id(nc)]

    import jax
    from jax.sharding import Mesh, NamedSharding, PartitionSpec

    from jax.experimental.shard_map import shard_map
    from concourse import mybir
    from concourse.bass2jax import (
        _bass_exec_p,
        install_neuronx_cc_hook,
        partition_id_tensor,
    )

    install_neuronx_cc_hook()
    partition_name = nc.partition_id_tensor.name if nc.partition_id_tensor else None

    in_names, out_names, out_avals = [], [], []
    for alloc in nc.m.functions[0].allocations:
        if not isinstance(alloc, mybir.MemoryLocationSet):
            continue
        name = alloc.memorylocations[0].name
        if alloc.kind == "ExternalInput":
            if name != partition_name:
                in_names.append(name)
        elif alloc.kind == "ExternalOutput":
            out_names.append(name)
            out_avals.append(
                jax.core.ShapedArray(
                    tuple(alloc.tensor_shape), mybir.dt.np(alloc.dtype)
                )
            )
    n_params = len(in_names)
    all_names = in_names + out_names
    if partition_name is not None:
        all_names = all_names + [partition_name]

    def _body(*args):
        operands = list(args)
        if partition_name is not None:
            operands.append(partition_id_tensor())
        return tuple(
            _bass_exec_p.bind(
                *operands,
                out_avals=tuple(out_avals),
                in_names=tuple(all_names),
                out_names=tuple(out_names),
                lowering_input_output_aliases=(),
                sim_require_finite=True,
                sim_require_nnan=True,
                nc=nc,
            )
        )

    devices = jax.devices()[:N_CORES]
    mesh = Mesh(np.asarray(devices), ("core",))
    sharded = jax.jit(
        shard_map(
            _body,
            mesh=mesh,
            in_specs=(PartitionSpec("core"),) * (n_params + len(out_names)),
            out_specs=(PartitionSpec("core"),) * len(out_names),
            check_rep=False,
        ),
        keep_unused=True,
    )
    sh = NamedSharding(mesh, PartitionSpec("core"))

    def run(in_maps):
        concat_in = [
            np.concatenate(
                [np.asarray(in_maps[c][nm]) for c in range(N_CORES)], axis=0
            )
            for nm in in_names
        ]
        concat_zero = [
            np.zeros((N_CORES * av.shape[0], *av.shape[1:]), av.dtype)
            for av in out_avals
        ]
        dev_args = [jax.device_put(a, sh) for a in concat_in + concat_zero]
        out_arrs = sharded(*dev_args)
        return [
            {
                nm: np.asarray(out_arrs[i]).reshape(
                    N_CORES, *out_avals[i].shape
                )[c]
                for i, nm in enumerate(out_names)
            }
            for c in range(N_CORES)
        ]

    _RUNNER_CACHE[id(nc)] = run
    return run


def kernel(x, mask, wq, wk, wv, wo, shape, scale, loc, start_pos):
    m2 = np.asarray(mask, np.float32).reshape(S, S)
    if _toeplitz_profile(m2) is None:
        return _numpy_fallback(
            x, mask, wq, wk, wv, wo, shape, scale, loc, start_pos
        )
    nc, in_maps = prepare(x, mask, wq, wk, wv, wo, shape, scale, loc, start_pos)

    if os.environ.get("KBA_SIM", "0") == "1":
        from concourse import bass_interp

        n_sim = int(os.environ.get("KBA_SIM_CORES", str(N_CORES)))
        sim = bass_interp.MultiCoreSim(nc, n_sim)
        for c in range(n_sim):
            for k, v in in_maps[c].items():
                sim.cores[c].tensor(k)[:] = v
        sim.simulate()
        results = [
            {"outT": np.array(sim.cores[c].tensor("outT"), np.float32)}
            for c in range(n_sim)
        ] + [
            {"outT": np.zeros((S, S), np.float32)} for _ in range(N_CORES - n_sim)
        ]
        LAST_RUN_INFO["exec_time_ns"] = None
    else:
        results = _get_runner(nc)(in_maps)
        LAST_RUN_INFO["exec_time_ns"] = None

    LAST_RUN_INFO["results"] = results
    return _reduce(results)


# revision 26
# speedup vs baseline: 1.2010x; 1.1440x over previous
"""BayesianAttention (ALiBi-style power-law prior + causal mask) on 8 trn2 cores.

Self-contained: builds a Bass/Tile kernel, shards heads across 8 NeuronCores
(2 heads per core; wq/wk/wv column-sharded, wo row-sharded), runs via a
shard_map'd bass program, and reduces the partial outputs on host.

v2 layout (all-bf16 operands, batched DMA, software-pipelined attention):
  host sends x^T [c, i] bf16; device computes q^T/k^T/v^T = W^T x^T in two
  PSUM passes (q,k then v), transposes v, s^T[j,i] = k^T_j . q^T_i,
  probs = exp(s^T) * EG  (EG = exp(prior + mask), a Toeplitz table indexed
  by j - i, precomputed on host, 0 where masked), o^T[d,i] = v^T probs with
  softmax sum via an all-ones stationary matmul, out^T[e,i] = wo^T (o^T/sum).
  Host returns sum_c(out^T_c)^T.

  All DMAs are batched (few large transfers) and issued on the SP queue;
  the attention inner loop is one global software pipeline across (ib, h)
  blocks so the PE never drains between blocks; phase-3 output rows are
  accumulated in SBUF and written once per 128-row stripe.
"""

import math
import os
from collections import deque

import ml_dtypes
import numpy as np

S = 2048          # sequence length
DIM = 2048        # model dim
H = 16            # heads
HD = 128          # head dim
N_CORES = 8
HL = H // N_CORES  # heads per core (2)
DL = HL * HD       # local projected dim (256)
IB = 512           # i-block (query block, moving free dim)
NIB = S // IB
NJT = S // 128     # key tiles of 128
NQ = 4             # x row-quads (512 rows each)
EPS = 1e-5
MASKED_THRESH = -1e8   # additive mask values below this mean "fully masked"

MM_DTYPE = os.environ.get("KBA_DTYPE", "bf16")  # "bf16" | "f32"
LAG = int(os.environ.get("KBA_LAG", "4"))  # scores->o-matmul emission lag
# every k-th eg-multiply runs on Pool instead of DVE (0 = all on DVE; Pool's
# low-efficiency multiply adds latency to the probs chain, so DVE-only wins)
MULPOOL = int(os.environ.get("KBA_MULPOOL", "0"))

LAST_RUN_INFO = {}


# ---------------------------------------------------------------- tile patch
def _apply_tile_patch():
    """walrus CoreV3 codegen tolerates only one sync-wait on an InstDrain;
    the tile-exit drain waits on the whole global clock. Spread the waits
    across extra SP nops."""
    import concourse.tile as tile
    from concourse import mybir
    from concourse.vector_clock import ScopedClock

    if getattr(tile.TileContext, "_kba_patched", False):
        return

    def _drain_and_barrier(self, tick_clock, wait_clock):
        nc = self.nc
        drain_inst = nc.sync.drain()
        wait_clock.add_sem_waits(
            drain_inst.ins, ScopedClock({None: tick_clock.global_clock})
        )
        si = drain_inst.ins.sync_info
        waits = list(si.on_wait or [])
        if len(waits) > 1:
            si.on_wait = waits[:1]
            for i in range(1, len(waits)):
                nop = nc.sync.nop(nofuse=True)
                nop.ins.sync_info = mybir.SyncInfo(
                    on_wait=waits[i : i + 1], on_update=[]
                )
        nc.all_engine_barrier()
        assert self.sems is not None
        popped = nc._tile_sem_poison_stack.pop()
        assert popped is self._sem_poison
        nc.clear_and_free_semaphores(list(self.sems.allocated().values()))
        nc.all_engine_barrier()

    tile.TileContext._drain_and_barrier = _drain_and_barrier
    tile.TileContext._kba_patched = True

    try:
        import concourse.tile_utils as tile_utils

        tile_utils.max_sbuf_usage = 208 * 1024
    except Exception:
        pass


# ------------------------------------------------------------- host helpers
def _toeplitz_profile(m2):
    """If mask[i, j] == phi(j - i) for all i,j, return phi (length 2S-1,
    index t + S - 1), else None."""
    phi = np.empty(2 * S - 1, dtype=np.float32)
    phi[S - 1 :] = m2[0, :]
    phi[: S - 1] = m2[1:, 0][::-1]
    idx = (np.arange(S)[None, :] - np.arange(S)[:, None]) + (S - 1)
    if np.array_equal(phi[idx], m2):
        return phi
    return None


# tiles/columns whose max prior weight is below this contribute nothing:
# host dry-run shows even element-granularity dropping at 1e-4 leaves the
# output bit-identical (errors move only at 1e-3)
EG_TAU = 1e-4


def _eg_profile(head, shape, scale, loc, start_pos, phi):
    """1-D prior weight profile eg1[d + (S-1)] = exp(prior(d) + phi(-d)) for
    d = i - j in [-(S-1), S-1]. The 2-D EG table is eg1 evaluated per (p,u);
    a tile's max EG is the max of eg1 over the tile's contiguous d-range."""
    d = np.arange(-(S - 1), S, dtype=np.int64)
    dist = (-d - start_pos).astype(np.float32)
    sh = np.float32(shape[0, head, 0, 0])
    sc = np.float32(scale[0, head, 0, 0])
    lo = np.float32(loc[0, head, 0, 0])
    loc_t = np.float32(np.exp(lo) - np.exp(-lo))
    z = (dist - loc_t) * np.exp(sc, dtype=np.float32)
    g = -np.power(np.abs(z) + np.float32(EPS), sh, dtype=np.float32)
    g = g + phi[np.clip(-d + (S - 1), 0, 2 * S - 2)]
    return np.exp(g)


def _head_kept_tiles(eg1):
    """kept[ib] = j-tiles where the head's prior x mask weight is non-negligible
    somewhere in the [IB x 128] block."""
    kept = []
    for ib in range(NIB):
        row = []
        for jt in range(NJT):
            dlo = ib * IB - jt * 128 - 127
            dhi = ib * IB + IB - 1 - jt * 128
            lo = max(dlo + (S - 1), 0)
            hi = min(dhi + (S - 1), 2 * S - 2)
            if lo <= hi and eg1[lo : hi + 1].max() >= EG_TAU:
                row.append(jt)
        kept.append(row)
    return kept


def _tile_support(eg_slot, ib, jt):
    """(off, w): the i-column range of block ib where tile jt has any
    non-negligible prior x mask weight. eg_slot is the per-slot max of the
    heads' 1-D profiles; column i_local covers d = (ib*IB + i_local) - j for
    j in the tile, a 128-wide window of eg_slot."""
    pad = np.zeros(2 * S - 1 + 256, np.float32)
    pad[128 : 128 + 2 * S - 1] = eg_slot
    # max over the 128-wide window ending at d = ib*IB + i - jt*128
    idx = (ib * IB + np.arange(IB) - jt * 128) + (S - 1) + 128
    win = np.lib.stride_tricks.sliding_window_view(pad, 128)
    colmax = win[idx - 127].max(axis=1)
    on = np.nonzero(colmax >= EG_TAU)[0]
    assert on.size, "kept tile with empty support"
    off = int(on[0]) & ~3
    end = min(IB, (int(on[-1]) + 1 + 3) & ~3)
    return off, end - off


def _eg_geometry(kept_slots):
    """The EG table covers u = base..base+IB for every kept (ib, jt), where
    base = ib*IB - jt*128 + (S-1). Returns (offset, width)."""
    bases = [
        ib * IB - jt * 128 + (S - 1)
        for kept in kept_slots
        for ib in range(NIB)
        for jt, _, _ in kept[ib]
    ]
    off = min(bases)
    width = max(bases) + IB - off
    return off, width


def _eg_table(head, shape, scale, loc, start_pos, phi, eg_off, eg_w):
    """[128, eg_w] float32: EG[p, u'] = exp(prior(d) + phi(-d)), 0 where
    masked/out of range, with d = i - j = (u' + eg_off) - p - (S - 1)."""
    p = np.arange(128, dtype=np.int64)[:, None]
    u = eg_off + np.arange(eg_w, dtype=np.int64)[None, :]
    d = u - p - (S - 1)          # i - j
    dist = (-d - start_pos).astype(np.float32)  # k_pos - q_pos
    sh = np.float32(shape[0, head, 0, 0])
    sc = np.float32(scale[0, head, 0, 0])
    lo = np.float32(loc[0, head, 0, 0])
    loc_t = np.float32(np.exp(lo) - np.exp(-lo))
    z = (dist - loc_t) * np.exp(sc, dtype=np.float32)
    g = -np.power(np.abs(z) + np.float32(EPS), sh, dtype=np.float32)
    t = np.clip(-d + (S - 1), 0, 2 * S - 2)
    g = g + phi[t]
    g[(-d < -(S - 1)) | (-d > (S - 1))] = -np.inf  # out of range: never read
    return np.ascontiguousarray(np.exp(g).astype(np.float32))


# ------------------------------------------------------------ program build
_PROGRAM_CACHE = {}


def _build_program(mm_name, kept_key, eg_off, eg_w, repeat=1):
    key = (mm_name, kept_key, eg_off, eg_w, repeat)
    if key in _PROGRAM_CACHE:
        return _PROGRAM_CACHE[key]

    import concourse.bass as bass
    import concourse.tile as tile
    from concourse import bacc, mybir
    from concourse.masks import make_identity

    _apply_tile_patch()

    f32 = mybir.dt.float32
    sdt = mybir.dt.bfloat16 if mm_name == "bf16" else f32

    # kept_key[h][ib] = j-tiles for local head slot h (slot 0 carries the
    # narrow-window heads, slot 1 the wide ones; identical across cores)
    kept = [[list(row) for row in slot] for slot in kept_key]

    nc = bacc.Bacc(
        "TRN2", target_bir_lowering=False, debug=False, num_devices=N_CORES
    )
    xT_d = nc.dram_tensor("xT", [S, S], sdt, kind="ExternalInput")
    wq_d = nc.dram_tensor("wq", [S, DL], sdt, kind="ExternalInput")
    wk_d = nc.dram_tensor("wk", [S, DL], sdt, kind="ExternalInput")
    wv_d = nc.dram_tensor("wv", [S, DL], sdt, kind="ExternalInput")
    wo_d = nc.dram_tensor("wo", [DL, S], sdt, kind="ExternalInput")
    eg_d = nc.dram_tensor("eg", [HL, 128, eg_w], sdt, kind="ExternalInput")
    outT_d = nc.dram_tensor("outT", [S, S], sdt, kind="ExternalOutput")

    Exp = mybir.ActivationFunctionType.Exp
    Copy = mybir.ActivationFunctionType.Copy

    with tile.TileContext(nc) as tc:
        import contextlib

        with contextlib.ExitStack() as ctx:
            consts = ctx.enter_context(tc.tile_pool(name="consts", bufs=1))
            persist = ctx.enter_context(tc.tile_pool(name="persist", bufs=1))
            xpool = ctx.enter_context(tc.tile_pool(name="xp", bufs=8))
            ppool = ctx.enter_context(tc.tile_pool(name="probs", bufs=4))
            vtpool = ctx.enter_context(tc.tile_pool(name="vt", bufs=2))
            rpool = ctx.enter_context(tc.tile_pool(name="rp", bufs=2))
            opool = ctx.enter_context(tc.tile_pool(name="orow", bufs=4))
            # PSUM: tag X (4 banks) = proj q/k accumulators | scores | out
            # tiles; tag Y (2 banks) = proj v accumulators | o-accumulators;
            # tag Z (2 banks) = v-transpose blocks | softmax-sum accumulators.
            psum = ctx.enter_context(tc.tile_pool(name="ps", bufs=2, space="PSUM"))

            # ---- persistent SBUF ----
            wq_sb = consts.tile([128, NJT, DL], sdt, name="wq_sb")
            wk_sb = consts.tile([128, NJT, DL], sdt, name="wk_sb")
            wv_sb = consts.tile([128, NJT, DL], sdt, name="wv_sb")
            wo_sb = consts.tile([128, HL, S], sdt, name="wo_sb")
            eg_sb = consts.tile([128, HL, eg_w], sdt, name="eg_sb")
            # [128, 128] all-ones stationary: the softmax-sum matmul then
            # produces Sum broadcast across all 128 partitions at no extra
            # PE cost (cycles scale with the moving width, not stationary m).
            ones_sb = consts.tile([128, 128], sdt, name="ones_sb")
            nc.vector.memset(ones_sb[:], 1.0)
            ident = consts.tile([128, 128], sdt, name="ident")
            make_identity(nc, ident[:])

            qT = persist.tile([128, HL, S], sdt)   # [d, h, i]
            kT = persist.tile([128, HL, S], sdt)   # [d, h, j]
            v_sb = persist.tile([128, HL, NJT, HD], sdt)  # [j, h, jt, d]
            o_sb = [
                persist.tile([128, HL, IB], sdt, name=f"o_sb{i}")
                for i in range(NIB)
            ]  # [d, h, i-block]

            for _rep in range(repeat):
                # ---- phase 1: projections (q^T, k^T, v^T), v transpose ----
                for ib in range(NIB):
                    isl = bass.ts(ib, IB)
                    xqs = []
                    for qd in range(NQ):
                        xq = xpool.tile([128, 4, IB], sdt, name="xq")
                        if _rep == 0 and ib == 0 and qd == 0:
                            # halve the very first transfers so the first
                            # matmul's operands land as early as possible
                            for hf in range(2):
                                cs = slice(hf * 2, hf * 2 + 2)
                                rs = slice(hf * 256, hf * 256 + 256)
                                # order: wq (first ldweights), x (first
                                # matmul), wk — minimizes the first stall
                                nc.sync.dma_start(
                                    out=wq_sb[:, cs, :],
                                    in_=wq_d[rs, :].rearrange(
                                        "(c p) d -> p c d", p=128
                                    ),
                                )
                                nc.sync.dma_start(
                                    out=xq[:, cs, :],
                                    in_=xT_d[rs, isl].rearrange(
                                        "(c p) i -> p c i", p=128
                                    ),
                                )
                                nc.sync.dma_start(
                                    out=wk_sb[:, cs, :],
                                    in_=wk_d[rs, :].rearrange(
                                        "(c p) d -> p c d", p=128
                                    ),
                                )
                            xqs.append(xq)
                            continue
                        nc.sync.dma_start(
                            out=xq[:],
                            in_=xT_d[qd * 512 : (qd + 1) * 512, isl].rearrange(
                                "(c p) i -> p c i", p=128
                            ),
                        )
                        xqs.append(xq)
                        if _rep == 0 and ib == 0:
                            # interleave weight quads with the first x quads
                            # so the first matmuls start ~3us in
                            for w_d, w_sb in ((wq_d, wq_sb), (wk_d, wk_sb)):
                                nc.sync.dma_start(
                                    out=w_sb[:, qd * 4 : (qd + 1) * 4, :],
                                    in_=w_d[
                                        qd * 512 : (qd + 1) * 512, :
                                    ].rearrange("(c p) d -> p c d", p=128),
                                )
                    if _rep == 0 and ib == 0:
                        for qd in range(NQ):
                            nc.sync.dma_start(
                                out=wv_sb[:, qd * 4 : (qd + 1) * 4, :],
                                in_=wv_d[
                                    qd * 512 : (qd + 1) * 512, :
                                ].rearrange("(c p) d -> p c d", p=128),
                            )
                    # pass A: q and k (4 PSUM banks, tag X)
                    psA = {}
                    for proj in range(2):
                        for dt_i in range(HL):
                            psA[(proj, dt_i)] = psum.tile(
                                [128, IB], f32, tag="X", bufs=4,
                                name=f"psA{proj}{dt_i}",
                            )
                    for ct in range(NJT):
                        xs = xqs[ct // 4][:, ct % 4, :]
                        for proj, w_sb in ((0, wq_sb), (1, wk_sb)):
                            for dt_i in range(HL):
                                nc.tensor.matmul(
                                    psA[(proj, dt_i)][:],
                                    lhsT=w_sb[:, ct, dt_i * HD : (dt_i + 1) * HD],
                                    rhs=xs,
                                    start=(ct == 0),
                                    stop=(ct == NJT - 1),
                                )
                    for dt_i in range(HL):
                        nc.scalar.activation(qT[:, dt_i, isl], psA[(0, dt_i)][:], Copy)
                        nc.vector.tensor_copy(kT[:, dt_i, isl], psA[(1, dt_i)][:])
                    # pass B: v (2 PSUM banks, tag Y)
                    psB = [
                        psum.tile([128, IB], f32, tag="Y", bufs=2, name=f"psB{d}")
                        for d in range(HL)
                    ]
                    for ct in range(NJT):
                        xs = xqs[ct // 4][:, ct % 4, :]
                        for dt_i in range(HL):
                            nc.tensor.matmul(
                                psB[dt_i][:],
                                lhsT=wv_sb[:, ct, dt_i * HD : (dt_i + 1) * HD],
                                rhs=xs,
                                start=(ct == 0),
                                stop=(ct == NJT - 1),
                            )
                    for dt_i in range(HL):
                        vt = vtpool.tile([128, IB], sdt, name="vt")
                        if dt_i == 0:
                            nc.vector.tensor_copy(vt[:], psB[dt_i][:])
                        else:
                            nc.scalar.activation(vt[:], psB[dt_i][:], Copy)
                        # transpose v^T [d, j] -> v [j, d] in 128-blocks
                        for s4 in range(IB // 128):
                            jt = (ib * IB) // 128 + s4
                            tp = psum.tile(
                                [128, 128], sdt, tag="Z", bufs=2, name="tp"
                            )
                            nc.tensor.transpose(
                                tp[:], vt[:, s4 * 128 : (s4 + 1) * 128], ident[:]
                            )
                            if s4 % 2 == 0:
                                nc.vector.tensor_copy(v_sb[:, dt_i, jt, :], tp[:])
                            else:
                                nc.scalar.activation(
                                    v_sb[:, dt_i, jt, :], tp[:], Copy
                                )

                if _rep == 0:
                    nc.sync.dma_start(
                        out=eg_sb[:], in_=eg_d.ap().rearrange("h p u -> p h u")
                    )
                    for h_i in range(HL):
                        nc.sync.dma_start(
                            out=wo_sb[:, h_i, :],
                            in_=wo_d[h_i * 128 : (h_i + 1) * 128, :],
                        )

                # ---- phase 2: attention, one global pipeline over blocks ----
                # Each tile computes only its prior-support columns [off, off+w)
                # of the i-block; the first tile of each accumulation group is
                # forced full-width so start=True zeroes the whole PSUM region.
                tasks = []
                for ib in range(NIB):
                    for h in range(HL):
                        jts = kept[h][ib]
                        for idx, (jt, off, w) in enumerate(jts):
                            if idx == 0:
                                off, w = 0, IB
                            tasks.append(
                                (ib, h, jt, off, w,
                                 idx == 0, idx == len(jts) - 1)
                            )

                block_acc = {}
                pend = deque()

                def emit_pv(t, pb):
                    ib, h, jt, off, w, first, last = t
                    if first:
                        block_acc[(ib, h)] = (
                            psum.tile([128, IB], f32, tag="Y", bufs=2, name="oacc"),
                            psum.tile([128, IB], f32, tag="Z", bufs=2, name="sacc"),
                        )
                    oacc, sacc = block_acc[(ib, h)]
                    nc.tensor.matmul(
                        oacc[:, off : off + w], lhsT=v_sb[:, h, jt, :],
                        rhs=pb[:, :w], start=first, stop=last,
                    )
                    nc.tensor.matmul(
                        sacc[:, off : off + w], lhsT=ones_sb[:],
                        rhs=pb[:, :w], start=first, stop=last,
                    )
                    if last:
                        rbc = rpool.tile([128, IB], f32, name="rbc")
                        nc.vector.reciprocal(rbc[:], sacc[:])
                        nc.vector.tensor_mul(o_sb[ib][:, h, :], oacc[:], rbc[:])

                for ti, t in enumerate(tasks):
                    ib, h, jt, off, w, first, last = t
                    sc = psum.tile([128, IB], f32, tag="X", bufs=4, name="sc")
                    nc.tensor.matmul(
                        sc[:, :w],
                        lhsT=kT[:, h, jt * 128 : (jt + 1) * 128],
                        rhs=qT[:, h, ib * IB + off : ib * IB + off + w],
                        start=True,
                        stop=True,
                    )
                    pb0 = ppool.tile([128, IB], sdt, tag="pb0", bufs=4, name="pb0")
                    nc.scalar.activation(pb0[:, :w], sc[:, :w], Exp)
                    pb = ppool.tile([128, IB], sdt, tag="pb", bufs=LAG + 2, name="pb")
                    base = ib * IB - jt * 128 + (S - 1) - eg_off + off
                    # eg multiply is SBUF->SBUF: offload a share to Pool
                    on_pool = MULPOOL > 0 and ti % MULPOOL == MULPOOL - 1
                    (nc.gpsimd if on_pool else nc.vector).tensor_mul(
                        pb[:, :w], pb0[:, :w], eg_sb[:, h, base : base + w]
                    )
                    pend.append((t, pb))
                    if len(pend) > LAG:
                        emit_pv(*pend.popleft())
                while pend:
                    emit_pv(*pend.popleft())

                # ---- phase 3: out^T = wo^T @ (o^T/sum); host sums cores ----
                for et in range(NJT):
                    orow = opool.tile([128, S], sdt, name="orow")
                    for ib in range(NIB):
                        po = psum.tile([128, IB], f32, tag="X", bufs=4, name="po")
                        for h in range(HL):
                            nc.tensor.matmul(
                                po[:],
                                lhsT=wo_sb[:, h, et * 128 : (et + 1) * 128],
                                rhs=o_sb[ib][:, h, :],
                                start=(h == 0),
                                stop=(h == HL - 1),
                            )
                        osl = bass.ts(ib, IB)
                        if (et * NIB + ib) % 2 == 0:
                            nc.scalar.activation(orow[:, osl], po[:], Copy)
                        else:
                            nc.vector.tensor_copy(orow[:, osl], po[:])
                    nc.sync.dma_start(
                        out=outT_d[et * 128 : (et + 1) * 128, :], in_=orow[:]
                    )

    nc.compile()
    _PROGRAM_CACHE[key] = nc
    return nc


# ------------------------------------------------------------------- kernel
def prepare(x, mask, wq, wk, wv, wo, shape, scale, loc, start_pos):
    """Host prep: build/cache program and per-core input maps."""
    mm_name = MM_DTYPE
    np_store = ml_dtypes.bfloat16 if mm_name == "bf16" else np.float32

    x32 = np.asarray(x, np.float32).reshape(S, DIM)
    m2 = np.asarray(mask, np.float32).reshape(S, S)
    wq32 = np.asarray(wq, np.float32)
    wk32 = np.asarray(wk, np.float32)
    wv32 = np.asarray(wv, np.float32)
    wo32 = np.asarray(wo, np.float32)
    shape = np.asarray(shape, np.float32)
    scale = np.asarray(scale, np.float32)
    loc = np.asarray(loc, np.float32)
    sp = int(start_pos)

    phi = _toeplitz_profile(m2)
    if phi is None:
        raise ValueError("non-Toeplitz mask: use _numpy_fallback")

    # Per-head kept tiles from the prior window; sort heads narrow->wide and
    # give slot 0 the 8 narrowest (the SPMD program computes the per-slot
    # union, so grouping similar windows minimizes wasted tiles).
    head_kept = []
    profiles = []
    for g in range(H):
        eg1 = _eg_profile(g, shape, scale, loc, sp, phi)
        profiles.append(eg1)
        head_kept.append(_head_kept_tiles(eg1))
    counts = [sum(len(r) for r in k) for k in head_kept]
    perm = list(np.argsort(np.asarray(counts), kind="stable"))
    kept_slots = []
    full_w = narrow_w = 0
    for s in range(HL):
        heads = perm[s * N_CORES : (s + 1) * N_CORES]
        eg_slot = np.maximum.reduce([profiles[g] for g in heads]).astype(
            np.float32
        )
        slot_rows = []
        for ib in range(NIB):
            jts = sorted(set().union(*[set(head_kept[g][ib]) for g in heads]))
            row = []
            for jt in jts:
                off, w = _tile_support(eg_slot, ib, jt)
                row.append((jt, off, w))
                full_w += IB
                narrow_w += w
            slot_rows.append(row)
        kept_slots.append(slot_rows)
    LAST_RUN_INFO["narrow_frac"] = narrow_w / max(full_w, 1)
    kept_key = tuple(
        tuple(tuple(row) for row in slot) for slot in kept_slots
    )
    eg_off, eg_w = _eg_geometry(kept_slots)

    LAST_RUN_INFO["build_args"] = (mm_name, kept_key, eg_off, eg_w)
    LAST_RUN_INFO["perm"] = perm
    nc = _build_program(mm_name, kept_key, eg_off, eg_w)

    xT = np.ascontiguousarray(x32.T).astype(np_store)
    inv_s = np.float32(1.0 / math.sqrt(HD))

    in_maps = []
    for c in range(N_CORES):
        heads = [perm[c], perm[N_CORES + c]]
        cols = np.concatenate(
            [np.arange(g * HD, (g + 1) * HD) for g in heads]
        )
        im = {
            "xT": xT,
            "wq": np.ascontiguousarray(wq32[:, cols] * inv_s).astype(np_store),
            "wk": np.ascontiguousarray(wk32[:, cols]).astype(np_store),
            "wv": np.ascontiguousarray(wv32[:, cols]).astype(np_store),
            "wo": np.ascontiguousarray(wo32[cols, :]).astype(np_store),
            "eg": np.stack(
                [
                    _eg_table(g, shape, scale, loc, sp, phi, eg_off, eg_w)
                    for g in heads
                ]
            ).astype(np_store),
        }
        in_maps.append(im)
    return nc, in_maps


def _numpy_fallback(x, mask, wq, wk, wv, wo, shape, scale, loc, start_pos):
    x2 = np.asarray(x, np.float32)[0]
    m = np.asarray(mask, np.float32)[0, 0]
    wq = np.asarray(wq, np.float32)
    wk = np.asarray(wk, np.float32)
    wv = np.asarray(wv, np.float32)
    wo = np.asarray(wo, np.float32)
    shape = np.asarray(shape, np.float32)
    scale = np.asarray(scale, np.float32)
    loc = np.asarray(loc, np.float32)
    sp = int(start_pos)
    q = (x2 @ wq).reshape(S, H, HD)
    k = (x2 @ wk).reshape(S, H, HD)
    v = (x2 @ wv).reshape(S, H, HD)
    out = np.zeros((S, H * HD), np.float32)
    qpos = np.arange(S, dtype=np.float32) + sp
    kpos = np.arange(S, dtype=np.float32)
    dist = kpos[None, :] - qpos[:, None]
    for h in range(H):
        s = (q[:, h] @ k[:, h].T) / np.float32(math.sqrt(HD))
        lo = loc[0, h, 0, 0]
        loc_t = np.exp(lo) - np.exp(-lo)
        z = (dist - loc_t) * np.exp(scale[0, h, 0, 0])
        s = s - (np.abs(z) + np.float32(EPS)) ** shape[0, h, 0, 0]
        s = s + m
        e = np.exp(s - s.max(axis=-1, keepdims=True))
        p = e / e.sum(axis=-1, keepdims=True)
        out[:, h * HD : (h + 1) * HD] = p @ v[:, h]
    return (out @ wo)[None].astype(np.float32)


def _reduce(results):
    acc = results[0]["outT"].astype(np.float32)
    for c in range(1, N_CORES):
        acc = acc + results[c]["outT"].astype(np.float32)
    return np.ascontiguousarray(acc.T)[None].astype(np.float32)


_RUNNER_CACHE = {}


def _get_runner(nc):
    """Build (once) a reusable jitted 8-core runner for the program `nc`.
    Mirrors bass2jax.run_bass_via_pjrt's multi-core path without output
    donation (outT is fully written by the kernel) so it can be re-invoked."""
    if id(nc) in _RUNNER_CACHE:
        return _RUNNER_CACHE[id(nc)]

    import jax
    from jax.sharding import Mesh, NamedSharding, PartitionSpec

    from jax.experimental.shard_map import shard_map
    from concourse import mybir
    from concourse.bass2jax import (
        _bass_exec_p,
        install_neuronx_cc_hook,
        partition_id_tensor,
    )

    install_neuronx_cc_hook()
    partition_name = nc.partition_id_tensor.name if nc.partition_id_tensor else None

    in_names, out_names, out_avals = [], [], []
    for alloc in nc.m.functions[0].allocations:
        if not isinstance(alloc, mybir.MemoryLocationSet):
            continue
        name = alloc.memorylocations[0].name
        if alloc.kind == "ExternalInput":
            if name != partition_name:
                in_names.append(name)
        elif alloc.kind == "ExternalOutput":
            out_names.append(name)
            out_avals.append(
                jax.core.ShapedArray(
                    tuple(alloc.tensor_shape), mybir.dt.np(alloc.dtype)
                )
            )
    n_params = len(in_names)
    all_names = in_names + out_names
    if partition_name is not None:
        all_names = all_names + [partition_name]

    def _body(*args):
        operands = list(args)
        if partition_name is not None:
            operands.append(partition_id_tensor())
        return tuple(
            _bass_exec_p.bind(
                *operands,
                out_avals=tuple(out_avals),
                in_names=tuple(all_names),
                out_names=tuple(out_names),
                lowering_input_output_aliases=(),
                sim_require_finite=True,
                sim_require_nnan=True,
                nc=nc,
            )
        )

    devices = jax.devices()[:N_CORES]
    mesh = Mesh(np.asarray(devices), ("core",))
    sharded = jax.jit(
        shard_map(
            _body,
            mesh=mesh,
            in_specs=(PartitionSpec("core"),) * (n_params + len(out_names)),
            out_specs=(PartitionSpec("core"),) * len(out_names),
            check_rep=False,
        ),
        keep_unused=True,
    )
    sh = NamedSharding(mesh, PartitionSpec("core"))

    def run(in_maps):
        concat_in = [
            np.concatenate(
                [np.asarray(in_maps[c][nm]) for c in range(N_CORES)], axis=0
            )
            for nm in in_names
        ]
        concat_zero = [
            np.zeros((N_CORES * av.shape[0], *av.shape[1:]), av.dtype)
            for av in out_avals
        ]
        dev_args = [jax.device_put(a, sh) for a in concat_in + concat_zero]
        out_arrs = sharded(*dev_args)
        return [
            {
                nm: np.asarray(out_arrs[i]).reshape(
                    N_CORES, *out_avals[i].shape
                )[c]
                for i, nm in enumerate(out_names)
            }
            for c in range(N_CORES)
        ]

    _RUNNER_CACHE[id(nc)] = run
    return run


def kernel(x, mask, wq, wk, wv, wo, shape, scale, loc, start_pos):
    m2 = np.asarray(mask, np.float32).reshape(S, S)
    if _toeplitz_profile(m2) is None:
        return _numpy_fallback(
            x, mask, wq, wk, wv, wo, shape, scale, loc, start_pos
        )
    nc, in_maps = prepare(x, mask, wq, wk, wv, wo, shape, scale, loc, start_pos)

    if os.environ.get("KBA_SIM", "0") == "1":
        from concourse import bass_interp

        n_sim = int(os.environ.get("KBA_SIM_CORES", str(N_CORES)))
        sim = bass_interp.MultiCoreSim(nc, n_sim)
        for c in range(n_sim):
            for k, v in in_maps[c].items():
                sim.cores[c].tensor(k)[:] = v
        sim.simulate()
        results = [
            {"outT": np.array(sim.cores[c].tensor("outT"), np.float32)}
            for c in range(n_sim)
        ] + [
            {"outT": np.zeros((S, S), np.float32)} for _ in range(N_CORES - n_sim)
        ]
        LAST_RUN_INFO["exec_time_ns"] = None
    else:
        results = _get_runner(nc)(in_maps)
        LAST_RUN_INFO["exec_time_ns"] = None

    LAST_RUN_INFO["results"] = results
    return _reduce(results)


# revision 28
# speedup vs baseline: 1.3639x; 1.1356x over previous
"""BayesianAttention (ALiBi-style power-law prior + causal mask) on 8 trn2 cores.

Self-contained: builds a Bass/Tile kernel, shards heads across 8 NeuronCores
(2 heads per core; wq/wk/wv column-sharded, wo row-sharded), runs via a
shard_map'd bass program, and reduces the partial outputs on host.

v2 layout (all-bf16 operands, batched DMA, software-pipelined attention):
  host sends x^T [c, i] bf16; device computes q^T/k^T/v^T = W^T x^T in two
  PSUM passes (q,k then v), transposes v, s^T[j,i] = k^T_j . q^T_i,
  probs = exp(s^T) * EG  (EG = exp(prior + mask), a Toeplitz table indexed
  by j - i, precomputed on host, 0 where masked), o^T[d,i] = v^T probs with
  softmax sum via an all-ones stationary matmul, out^T[e,i] = wo^T (o^T/sum).
  Host returns sum_c(out^T_c)^T.

  All DMAs are batched (few large transfers) and issued on the SP queue;
  the attention inner loop is one global software pipeline across (ib, h)
  blocks so the PE never drains between blocks; phase-3 output rows are
  accumulated in SBUF and written once per 128-row stripe.
"""

import math
import os
from collections import deque

import ml_dtypes
import numpy as np

S = 2048          # sequence length
DIM = 2048        # model dim
H = 16            # heads
HD = 128          # head dim
N_CORES = 8
HL = H // N_CORES  # heads per core (2)
DL = HL * HD       # local projected dim (256)
IB = 512           # i-block (query block, moving free dim)
NIB = S // IB
NJT = S // 128     # key tiles of 128
NQ = 4             # x row-quads (512 rows each)
EPS = 1e-5
MASKED_THRESH = -1e8   # additive mask values below this mean "fully masked"

MM_DTYPE = os.environ.get("KBA_DTYPE", "bf16")  # "bf16" | "f32"
LAG = int(os.environ.get("KBA_LAG", "4"))  # scores->o-matmul emission lag
# every k-th eg-multiply runs on Pool instead of DVE (0 = all on DVE; Pool's
# low-efficiency multiply adds latency to the probs chain, so DVE-only wins)
MULPOOL = int(os.environ.get("KBA_MULPOOL", "0"))

LAST_RUN_INFO = {}


# ---------------------------------------------------------------- tile patch
def _apply_tile_patch():
    """walrus CoreV3 codegen tolerates only one sync-wait on an InstDrain;
    the tile-exit drain waits on the whole global clock. Spread the waits
    across extra SP nops."""
    import concourse.tile as tile
    from concourse import mybir
    from concourse.vector_clock import ScopedClock

    if getattr(tile.TileContext, "_kba_patched", False):
        return

    def _drain_and_barrier(self, tick_clock, wait_clock):
        nc = self.nc
        drain_inst = nc.sync.drain()
        wait_clock.add_sem_waits(
            drain_inst.ins, ScopedClock({None: tick_clock.global_clock})
        )
        si = drain_inst.ins.sync_info
        waits = list(si.on_wait or [])
        if len(waits) > 1:
            si.on_wait = waits[:1]
            for i in range(1, len(waits)):
                nop = nc.sync.nop(nofuse=True)
                nop.ins.sync_info = mybir.SyncInfo(
                    on_wait=waits[i : i + 1], on_update=[]
                )
        nc.all_engine_barrier()
        assert self.sems is not None
        popped = nc._tile_sem_poison_stack.pop()
        assert popped is self._sem_poison
        nc.clear_and_free_semaphores(list(self.sems.allocated().values()))
        nc.all_engine_barrier()

    tile.TileContext._drain_and_barrier = _drain_and_barrier
    tile.TileContext._kba_patched = True

    try:
        import concourse.tile_utils as tile_utils

        tile_utils.max_sbuf_usage = 208 * 1024
    except Exception:
        pass


# ------------------------------------------------------------- host helpers
def _toeplitz_profile(m2):
    """If mask[i, j] == phi(j - i) for all i,j, return phi (length 2S-1,
    index t + S - 1), else None."""
    phi = np.empty(2 * S - 1, dtype=np.float32)
    phi[S - 1 :] = m2[0, :]
    phi[: S - 1] = m2[1:, 0][::-1]
    idx = (np.arange(S)[None, :] - np.arange(S)[:, None]) + (S - 1)
    if np.array_equal(phi[idx], m2):
        return phi
    return None


# tiles/columns whose max prior weight is below this contribute nothing:
# host dry-run shows even element-granularity dropping at 1e-4 leaves the
# output bit-identical (errors move only at 1e-3)
EG_TAU = 1e-4


def _eg_profile(head, shape, scale, loc, start_pos, phi):
    """1-D prior weight profile eg1[d + (S-1)] = exp(prior(d) + phi(-d)) for
    d = i - j in [-(S-1), S-1]. The 2-D EG table is eg1 evaluated per (p,u);
    a tile's max EG is the max of eg1 over the tile's contiguous d-range."""
    d = np.arange(-(S - 1), S, dtype=np.int64)
    dist = (-d - start_pos).astype(np.float32)
    sh = np.float32(shape[0, head, 0, 0])
    sc = np.float32(scale[0, head, 0, 0])
    lo = np.float32(loc[0, head, 0, 0])
    loc_t = np.float32(np.exp(lo) - np.exp(-lo))
    z = (dist - loc_t) * np.exp(sc, dtype=np.float32)
    g = -np.power(np.abs(z) + np.float32(EPS), sh, dtype=np.float32)
    g = g + phi[np.clip(-d + (S - 1), 0, 2 * S - 2)]
    return np.exp(g)


def _head_kept_tiles(eg1):
    """kept[ib] = j-tiles where the head's prior x mask weight is non-negligible
    somewhere in the [IB x 128] block."""
    kept = []
    for ib in range(NIB):
        row = []
        for jt in range(NJT):
            dlo = ib * IB - jt * 128 - 127
            dhi = ib * IB + IB - 1 - jt * 128
            lo = max(dlo + (S - 1), 0)
            hi = min(dhi + (S - 1), 2 * S - 2)
            if lo <= hi and eg1[lo : hi + 1].max() >= EG_TAU:
                row.append(jt)
        kept.append(row)
    return kept


def _tile_support(eg_slot, ib, jt):
    """(off, w): the i-column range of block ib where tile jt has any
    non-negligible prior x mask weight. eg_slot is the per-slot max of the
    heads' 1-D profiles; column i_local covers d = (ib*IB + i_local) - j for
    j in the tile, a 128-wide window of eg_slot."""
    pad = np.zeros(2 * S - 1 + 256, np.float32)
    pad[128 : 128 + 2 * S - 1] = eg_slot
    # max over the 128-wide window ending at d = ib*IB + i - jt*128
    idx = (ib * IB + np.arange(IB) - jt * 128) + (S - 1) + 128
    win = np.lib.stride_tricks.sliding_window_view(pad, 128)
    colmax = win[idx - 127].max(axis=1)
    on = np.nonzero(colmax >= EG_TAU)[0]
    assert on.size, "kept tile with empty support"
    off = int(on[0]) & ~3
    end = min(IB, (int(on[-1]) + 1 + 3) & ~3)
    return off, end - off


def _eg_geometry(kept_slots):
    """The EG table covers u = base..base+IB for every kept (ib, jt), where
    base = ib*IB - jt*128 + (S-1). Returns (offset, width)."""
    bases = [
        ib * IB - jt * 128 + (S - 1)
        for kept in kept_slots
        for ib in range(NIB)
        for jt, _, _ in kept[ib]
    ]
    off = min(bases)
    width = max(bases) + IB - off
    return off, width


def _eg_table(head, shape, scale, loc, start_pos, phi, eg_off, eg_w):
    """[128, eg_w] float32: EG[p, u'] = exp(prior(d) + phi(-d)), 0 where
    masked/out of range, with d = i - j = (u' + eg_off) - p - (S - 1)."""
    p = np.arange(128, dtype=np.int64)[:, None]
    u = eg_off + np.arange(eg_w, dtype=np.int64)[None, :]
    d = u - p - (S - 1)          # i - j
    dist = (-d - start_pos).astype(np.float32)  # k_pos - q_pos
    sh = np.float32(shape[0, head, 0, 0])
    sc = np.float32(scale[0, head, 0, 0])
    lo = np.float32(loc[0, head, 0, 0])
    loc_t = np.float32(np.exp(lo) - np.exp(-lo))
    z = (dist - loc_t) * np.exp(sc, dtype=np.float32)
    g = -np.power(np.abs(z) + np.float32(EPS), sh, dtype=np.float32)
    t = np.clip(-d + (S - 1), 0, 2 * S - 2)
    g = g + phi[t]
    g[(-d < -(S - 1)) | (-d > (S - 1))] = -np.inf  # out of range: never read
    return np.ascontiguousarray(np.exp(g).astype(np.float32))


# ------------------------------------------------------------ program build
_PROGRAM_CACHE = {}


def _build_program(mm_name, kept_key, eg_off, eg_w, repeat=1):
    key = (mm_name, kept_key, eg_off, eg_w, repeat)
    if key in _PROGRAM_CACHE:
        return _PROGRAM_CACHE[key]

    import concourse.bass as bass
    import concourse.tile as tile
    from concourse import bacc, mybir
    from concourse.masks import make_identity

    _apply_tile_patch()

    f32 = mybir.dt.float32
    sdt = mybir.dt.bfloat16 if mm_name == "bf16" else f32

    # kept_key[h][ib] = j-tiles for local head slot h (slot 0 carries the
    # narrow-window heads, slot 1 the wide ones; identical across cores)
    kept = [[list(row) for row in slot] for slot in kept_key]

    nc = bacc.Bacc(
        "TRN2", target_bir_lowering=False, debug=False, num_devices=N_CORES
    )
    xT_d = nc.dram_tensor("xT", [S, S], sdt, kind="ExternalInput")
    wq_d = nc.dram_tensor("wq", [S, DL], sdt, kind="ExternalInput")
    wk_d = nc.dram_tensor("wk", [S, DL], sdt, kind="ExternalInput")
    wv_d = nc.dram_tensor("wv", [S, DL], sdt, kind="ExternalInput")
    wo_d = nc.dram_tensor("wo", [DL, S], sdt, kind="ExternalInput")
    eg_d = nc.dram_tensor("eg", [HL, 128, eg_w], sdt, kind="ExternalInput")
    outT_d = nc.dram_tensor("outT", [S, S], sdt, kind="ExternalOutput")

    Exp = mybir.ActivationFunctionType.Exp
    Copy = mybir.ActivationFunctionType.Copy

    with tile.TileContext(nc) as tc:
        import contextlib

        with contextlib.ExitStack() as ctx:
            consts = ctx.enter_context(tc.tile_pool(name="consts", bufs=1))
            persist = ctx.enter_context(tc.tile_pool(name="persist", bufs=1))
            xpool = ctx.enter_context(tc.tile_pool(name="xp", bufs=8))
            ppool = ctx.enter_context(tc.tile_pool(name="probs", bufs=4))
            vtpool = ctx.enter_context(tc.tile_pool(name="vt", bufs=2))
            rpool = ctx.enter_context(tc.tile_pool(name="rp", bufs=2))
            opool = ctx.enter_context(tc.tile_pool(name="orow", bufs=4))
            # PSUM: tag X (4 banks) = proj q/k accumulators | scores | out
            # tiles; tag Y (2 banks) = proj v accumulators | o-accumulators;
            # tag Z (2 banks) = v-transpose blocks | softmax-sum accumulators.
            psum = ctx.enter_context(tc.tile_pool(name="ps", bufs=2, space="PSUM"))

            # ---- persistent SBUF ----
            wq_sb = consts.tile([128, NJT, DL], sdt, name="wq_sb")
            wk_sb = consts.tile([128, NJT, DL], sdt, name="wk_sb")
            wv_sb = consts.tile([128, NJT, DL], sdt, name="wv_sb")
            wo_sb = consts.tile([128, HL, S], sdt, name="wo_sb")
            eg_sb = consts.tile([128, HL, eg_w], sdt, name="eg_sb")
            # [128, 128] all-ones stationary: the softmax-sum matmul then
            # produces Sum broadcast across all 128 partitions at no extra
            # PE cost (cycles scale with the moving width, not stationary m).
            ones_sb = consts.tile([128, 128], sdt, name="ones_sb")
            nc.vector.memset(ones_sb[:], 1.0)
            ident = consts.tile([128, 128], sdt, name="ident")
            make_identity(nc, ident[:])

            qT = persist.tile([128, HL, S], sdt)   # [d, h, i]
            kT = persist.tile([128, HL, S], sdt)   # [d, h, j]
            v_sb = persist.tile([128, HL, NJT, HD], sdt)  # [j, h, jt, d]
            o_sb = [
                persist.tile([128, HL, IB], sdt, name=f"o_sb{i}")
                for i in range(NIB)
            ]  # [d, h, i-block]

            for _rep in range(repeat):
                # ---- phase 1: projections (q^T, k^T, v^T), v transpose ----
                for ib in range(NIB):
                    isl = bass.ts(ib, IB)
                    xqs = []
                    for qd in range(NQ):
                        xq = xpool.tile([128, 4, IB], sdt, name="xq")
                        if _rep == 0 and ib == 0 and qd == 0:
                            # halve the very first transfers so the first
                            # matmul's operands land as early as possible
                            for hf in range(2):
                                cs = slice(hf * 2, hf * 2 + 2)
                                rs = slice(hf * 256, hf * 256 + 256)
                                # order: wq (first ldweights), x (first
                                # matmul), wk — minimizes the first stall
                                nc.sync.dma_start(
                                    out=wq_sb[:, cs, :],
                                    in_=wq_d[rs, :].rearrange(
                                        "(c p) d -> p c d", p=128
                                    ),
                                )
                                nc.sync.dma_start(
                                    out=xq[:, cs, :],
                                    in_=xT_d[rs, isl].rearrange(
                                        "(c p) i -> p c i", p=128
                                    ),
                                )
                                nc.sync.dma_start(
                                    out=wk_sb[:, cs, :],
                                    in_=wk_d[rs, :].rearrange(
                                        "(c p) d -> p c d", p=128
                                    ),
                                )
                            xqs.append(xq)
                            continue
                        nc.sync.dma_start(
                            out=xq[:],
                            in_=xT_d[qd * 512 : (qd + 1) * 512, isl].rearrange(
                                "(c p) i -> p c i", p=128
                            ),
                        )
                        xqs.append(xq)
                        if _rep == 0 and ib == 0:
                            # interleave weight quads with the first x quads
                            # so the first matmuls start ~3us in
                            for w_d, w_sb in ((wq_d, wq_sb), (wk_d, wk_sb)):
                                nc.sync.dma_start(
                                    out=w_sb[:, qd * 4 : (qd + 1) * 4, :],
                                    in_=w_d[
                                        qd * 512 : (qd + 1) * 512, :
                                    ].rearrange("(c p) d -> p c d", p=128),
                                )
                    if _rep == 0 and ib == 0:
                        for qd in range(NQ):
                            nc.sync.dma_start(
                                out=wv_sb[:, qd * 4 : (qd + 1) * 4, :],
                                in_=wv_d[
                                    qd * 512 : (qd + 1) * 512, :
                                ].rearrange("(c p) d -> p c d", p=128),
                            )
                    # pass A: q and k (4 PSUM banks, tag X)
                    psA = {}
                    for proj in range(2):
                        for dt_i in range(HL):
                            psA[(proj, dt_i)] = psum.tile(
                                [128, IB], f32, tag="X", bufs=4,
                                name=f"psA{proj}{dt_i}",
                            )
                    for ct in range(NJT):
                        xs = xqs[ct // 4][:, ct % 4, :]
                        for proj, w_sb in ((0, wq_sb), (1, wk_sb)):
                            for dt_i in range(HL):
                                nc.tensor.matmul(
                                    psA[(proj, dt_i)][:],
                                    lhsT=w_sb[:, ct, dt_i * HD : (dt_i + 1) * HD],
                                    rhs=xs,
                                    start=(ct == 0),
                                    stop=(ct == NJT - 1),
                                )
                    for dt_i in range(HL):
                        nc.scalar.activation(qT[:, dt_i, isl], psA[(0, dt_i)][:], Copy)
                        nc.vector.tensor_copy(kT[:, dt_i, isl], psA[(1, dt_i)][:])
                    # pass B: v (2 PSUM banks, tag Y)
                    psB = [
                        psum.tile([128, IB], f32, tag="Y", bufs=2, name=f"psB{d}")
                        for d in range(HL)
                    ]
                    for ct in range(NJT):
                        xs = xqs[ct // 4][:, ct % 4, :]
                        for dt_i in range(HL):
                            nc.tensor.matmul(
                                psB[dt_i][:],
                                lhsT=wv_sb[:, ct, dt_i * HD : (dt_i + 1) * HD],
                                rhs=xs,
                                start=(ct == 0),
                                stop=(ct == NJT - 1),
                            )
                    for dt_i in range(HL):
                        vt = vtpool.tile([128, IB], sdt, name="vt")
                        if dt_i == 0:
                            nc.vector.tensor_copy(vt[:], psB[dt_i][:])
                        else:
                            nc.scalar.activation(vt[:], psB[dt_i][:], Copy)
                        # transpose v^T [d, j] -> v [j, d] in 128-blocks
                        for s4 in range(IB // 128):
                            jt = (ib * IB) // 128 + s4
                            tp = psum.tile(
                                [128, 128], sdt, tag="Z", bufs=2, name="tp"
                            )
                            nc.tensor.transpose(
                                tp[:], vt[:, s4 * 128 : (s4 + 1) * 128], ident[:]
                            )
                            if s4 % 2 == 0:
                                nc.vector.tensor_copy(v_sb[:, dt_i, jt, :], tp[:])
                            else:
                                nc.scalar.activation(
                                    v_sb[:, dt_i, jt, :], tp[:], Copy
                                )

                if _rep == 0:
                    nc.sync.dma_start(
                        out=eg_sb[:], in_=eg_d.ap().rearrange("h p u -> p h u")
                    )
                    for h_i in range(HL):
                        nc.sync.dma_start(
                            out=wo_sb[:, h_i, :],
                            in_=wo_d[h_i * 128 : (h_i + 1) * 128, :],
                        )

                # ---- phase 2: attention, one global pipeline over blocks ----
                # Each tile computes only its prior-support columns [off, off+w)
                # of the i-block; the first tile of each accumulation group is
                # forced full-width so start=True zeroes the whole PSUM region.
                tasks = []
                for ib in range(NIB):
                    for h in range(HL):
                        jts = kept[h][ib]
                        for idx, (jt, off, w) in enumerate(jts):
                            if idx == 0:
                                off, w = 0, IB
                            tasks.append(
                                (ib, h, jt, off, w,
                                 idx == 0, idx == len(jts) - 1)
                            )

                block_acc = {}
                pend = deque()

                def emit_pv(t, pb):
                    ib, h, jt, off, w, first, last = t
                    if first:
                        block_acc[(ib, h)] = (
                            psum.tile([128, IB], f32, tag="Y", bufs=2, name="oacc"),
                            psum.tile([128, IB], f32, tag="Z", bufs=2, name="sacc"),
                        )
                    oacc, sacc = block_acc[(ib, h)]
                    nc.tensor.matmul(
                        oacc[:, off : off + w], lhsT=v_sb[:, h, jt, :],
                        rhs=pb[:, :w], start=first, stop=last,
                    )
                    nc.tensor.matmul(
                        sacc[:, off : off + w], lhsT=ones_sb[:],
                        rhs=pb[:, :w], start=first, stop=last,
                    )
                    if last:
                        rbc = rpool.tile([128, IB], f32, name="rbc")
                        nc.vector.reciprocal(rbc[:], sacc[:])
                        nc.vector.tensor_mul(o_sb[ib][:, h, :], oacc[:], rbc[:])

                for ti, t in enumerate(tasks):
                    ib, h, jt, off, w, first, last = t
                    sc = psum.tile([128, IB], f32, tag="X", bufs=4, name="sc")
                    nc.tensor.matmul(
                        sc[:, :w],
                        lhsT=kT[:, h, jt * 128 : (jt + 1) * 128],
                        rhs=qT[:, h, ib * IB + off : ib * IB + off + w],
                        start=True,
                        stop=True,
                    )
                    pb0 = ppool.tile([128, IB], sdt, tag="pb0", bufs=4, name="pb0")
                    nc.scalar.activation(pb0[:, :w], sc[:, :w], Exp)
                    pb = ppool.tile([128, IB], sdt, tag="pb", bufs=LAG + 2, name="pb")
                    base = ib * IB - jt * 128 + (S - 1) - eg_off + off
                    # eg multiply is SBUF->SBUF: offload a share to Pool
                    on_pool = MULPOOL > 0 and ti % MULPOOL == MULPOOL - 1
                    (nc.gpsimd if on_pool else nc.vector).tensor_mul(
                        pb[:, :w], pb0[:, :w], eg_sb[:, h, base : base + w]
                    )
                    pend.append((t, pb))
                    if len(pend) > LAG:
                        emit_pv(*pend.popleft())
                while pend:
                    emit_pv(*pend.popleft())

                # ---- phase 3: out^T = wo^T @ (o^T/sum); host sums cores ----
                for et in range(NJT):
                    orow = opool.tile([128, S], sdt, name="orow")
                    for ib in range(NIB):
                        po = psum.tile([128, IB], f32, tag="X", bufs=4, name="po")
                        for h in range(HL):
                            nc.tensor.matmul(
                                po[:],
                                lhsT=wo_sb[:, h, et * 128 : (et + 1) * 128],
                                rhs=o_sb[ib][:, h, :],
                                start=(h == 0),
                                stop=(h == HL - 1),
                            )
                        osl = bass.ts(ib, IB)
                        if (et * NIB + ib) % 2 == 0:
                            nc.scalar.activation(orow[:, osl], po[:], Copy)
                        else:
                            nc.vector.tensor_copy(orow[:, osl], po[:])
                    nc.sync.dma_start(
                        out=outT_d[et * 128 : (et + 1) * 128, :], in_=orow[:]
                    )

    nc.compile()
    _PROGRAM_CACHE[key] = nc
    return nc


# ------------------------------------------------------------------- kernel
def prepare(x, mask, wq, wk, wv, wo, shape, scale, loc, start_pos):
    """Host prep: build/cache program and per-core input maps."""
    mm_name = MM_DTYPE
    np_store = ml_dtypes.bfloat16 if mm_name == "bf16" else np.float32

    x32 = np.asarray(x, np.float32).reshape(S, DIM)
    m2 = np.asarray(mask, np.float32).reshape(S, S)
    wq32 = np.asarray(wq, np.float32)
    wk32 = np.asarray(wk, np.float32)
    wv32 = np.asarray(wv, np.float32)
    wo32 = np.asarray(wo, np.float32)
    shape = np.asarray(shape, np.float32)
    scale = np.asarray(scale, np.float32)
    loc = np.asarray(loc, np.float32)
    sp = int(start_pos)

    phi = _toeplitz_profile(m2)
    if phi is None:
        raise ValueError("non-Toeplitz mask: use _numpy_fallback")

    # Per-head kept tiles from the prior window; sort heads narrow->wide and
    # give slot 0 the 8 narrowest (the SPMD program computes the per-slot
    # union, so grouping similar windows minimizes wasted tiles).
    head_kept = []
    profiles = []
    for g in range(H):
        eg1 = _eg_profile(g, shape, scale, loc, sp, phi)
        profiles.append(eg1)
        head_kept.append(_head_kept_tiles(eg1))
    counts = [sum(len(r) for r in k) for k in head_kept]
    perm = list(np.argsort(np.asarray(counts), kind="stable"))
    kept_slots = []
    full_w = narrow_w = 0
    for s in range(HL):
        heads = perm[s * N_CORES : (s + 1) * N_CORES]
        eg_slot = np.maximum.reduce([profiles[g] for g in heads]).astype(
            np.float32
        )
        slot_rows = []
        for ib in range(NIB):
            jts = sorted(set().union(*[set(head_kept[g][ib]) for g in heads]))
            row = []
            for jt in jts:
                off, w = _tile_support(eg_slot, ib, jt)
                row.append((jt, off, w))
                full_w += IB
                narrow_w += w
            slot_rows.append(row)
        kept_slots.append(slot_rows)
    LAST_RUN_INFO["narrow_frac"] = narrow_w / max(full_w, 1)
    kept_key = tuple(
        tuple(tuple(row) for row in slot) for slot in kept_slots
    )
    eg_off, eg_w = _eg_geometry(kept_slots)

    LAST_RUN_INFO["build_args"] = (mm_name, kept_key, eg_off, eg_w)
    LAST_RUN_INFO["perm"] = perm
    nc = _build_program(mm_name, kept_key, eg_off, eg_w)

    xT = np.ascontiguousarray(x32.T).astype(np_store)
    inv_s = np.float32(1.0 / math.sqrt(HD))

    in_maps = []
    for c in range(N_CORES):
        heads = [perm[c], perm[N_CORES + c]]
        cols = np.concatenate(
            [np.arange(g * HD, (g + 1) * HD) for g in heads]
        )
        im = {
            "xT": xT,
            "wq": np.ascontiguousarray(wq32[:, cols] * inv_s).astype(np_store),
            "wk": np.ascontiguousarray(wk32[:, cols]).astype(np_store),
            "wv": np.ascontiguousarray(wv32[:, cols]).astype(np_store),
            "wo": np.ascontiguousarray(wo32[cols, :]).astype(np_store),
            "eg": np.stack(
                [
                    _eg_table(g, shape, scale, loc, sp, phi, eg_off, eg_w)
                    for g in heads
                ]
            ).astype(np_store),
        }
        in_maps.append(im)
    return nc, in_maps


def _numpy_fallback(x, mask, wq, wk, wv, wo, shape, scale, loc, start_pos):
    x2 = np.asarray(x, np.float32)[0]
    m = np.asarray(mask, np.float32)[0, 0]
    wq = np.asarray(wq, np.float32)
    wk = np.asarray(wk, np.float32)
    wv = np.asarray(wv, np.float32)
    wo = np.asarray(wo, np.float32)
    shape = np.asarray(shape, np.float32)
    scale = np.asarray(scale, np.float32)
    loc = np.asarray(loc, np.float32)
    sp = int(start_pos)
    q = (x2 @ wq).reshape(S, H, HD)
    k = (x2 @ wk).reshape(S, H, HD)
    v = (x2 @ wv).reshape(S, H, HD)
    out = np.zeros((S, H * HD), np.float32)
    qpos = np.arange(S, dtype=np.float32) + sp
    kpos = np.arange(S, dtype=np.float32)
    dist = kpos[None, :] - qpos[:, None]
    for h in range(H):
        s = (q[:, h] @ k[:, h].T) / np.float32(math.sqrt(HD))
        lo = loc[0, h, 0, 0]
        loc_t = np.exp(lo) - np.exp(-lo)
        z = (dist - loc_t) * np.exp(scale[0, h, 0, 0])
        s = s - (np.abs(z) + np.float32(EPS)) ** shape[0, h, 0, 0]
        s = s + m
        e = np.exp(s - s.max(axis=-1, keepdims=True))
        p = e / e.sum(axis=-1, keepdims=True)
        out[:, h * HD : (h + 1) * HD] = p @ v[:, h]
    return (out @ wo)[None].astype(np.float32)


def _reduce(results):
    acc = results[0]["outT"].astype(np.float32)
    for c in range(1, N_CORES):
        acc = acc + results[c]["outT"].astype(np.float32)
    return np.ascontiguousarray(acc.T)[None].astype(np.float32)


_RUNNER_CACHE = {}


def _get_runner(nc):
    """Build (once) a reusable jitted 8-core runner for the program `nc`.
    Mirrors bass2jax.run_bass_via_pjrt's multi-core path without output
    donation (outT is fully written by the kernel) so it can be re-invoked."""
    if id(nc) in _RUNNER_CACHE:
        return _RUNNER_CACHE[id(nc)]

    import jax
    from jax.sharding import Mesh, NamedSharding, PartitionSpec

    from jax.experimental.shard_map import shard_map
    from concourse import mybir
    from concourse.bass2jax import (
        _bass_exec_p,
        install_neuronx_cc_hook,
        partition_id_tensor,
    )

    install_neuronx_cc_hook()
    partition_name = nc.partition_id_tensor.name if nc.partition_id_tensor else None

    in_names, out_names, out_avals = [], [], []
    for alloc in nc.m.functions[0].allocations:
        if not isinstance(alloc, mybir.MemoryLocationSet):
            continue
        name = alloc.memorylocations[0].name
        if alloc.kind == "ExternalInput":
            if name != partition_name:
                in_names.append(name)
        elif alloc.kind == "ExternalOutput":
            out_names.append(name)
            out_avals.append(
                jax.core.ShapedArray(
                    tuple(alloc.tensor_shape), mybir.dt.np(alloc.dtype)
                )
            )
    n_params = len(in_names)
    all_names = in_names + out_names
    if partition_name is not None:
        all_names = all_names + [partition_name]

    def _body(*args):
        operands = list(args)
        if partition_name is not None:
            operands.append(partition_id_tensor())
        return tuple(
            _bass_exec_p.bind(
                *operands,
                out_avals=tuple(out_avals),
                in_names=tuple(all_names),
                out_names=tuple(out_names),
                lowering_input_output_aliases=(),
                sim_require_finite=True,
                sim_require_nnan=True,
                nc=nc,
            )
        )

    devices = jax.devices()[:N_CORES]
    mesh = Mesh(np.asarray(devices), ("core",))
    sharded = jax.jit(
        shard_map(
            _body,
            mesh=mesh,
            in_specs=(PartitionSpec("core"),) * (n_params + len(out_names)),
            out_specs=(PartitionSpec("core"),) * len(out_names),
            check_rep=False,
        ),
        keep_unused=True,
    )
    sh = NamedSharding(mesh, PartitionSpec("core"))

    def run(in_maps):
        concat_in = [
            np.concatenate(
                [np.asarray(in_maps[c][nm]) for c in range(N_CORES)], axis=0
            )
            for nm in in_names
        ]
        concat_zero = [
            np.zeros((N_CORES * av.shape[0], *av.shape[1:]), av.dtype)
            for av in out_avals
        ]
        dev_args = [jax.device_put(a, sh) for a in concat_in + concat_zero]
        out_arrs = sharded(*dev_args)
        return [
            {
                nm: np.asarray(out_arrs[i]).reshape(
                    N_CORES, *out_avals[i].shape
                )[c]
                for i, nm in enumerate(out_names)
            }
            for c in range(N_CORES)
        ]

    _RUNNER_CACHE[id(nc)] = run
    return run


def kernel(x, mask, wq, wk, wv, wo, shape, scale, loc, start_pos):
    m2 = np.asarray(mask, np.float32).reshape(S, S)
    if _toeplitz_profile(m2) is None:
        return _numpy_fallback(
            x, mask, wq, wk, wv, wo, shape, scale, loc, start_pos
        )
    nc, in_maps = prepare(x, mask, wq, wk, wv, wo, shape, scale, loc, start_pos)

    if os.environ.get("KBA_SIM", "0") == "1":
        from concourse import bass_interp

        n_sim = int(os.environ.get("KBA_SIM_CORES", str(N_CORES)))
        sim = bass_interp.MultiCoreSim(nc, n_sim)
        for c in range(n_sim):
            for k, v in in_maps[c].items():
                sim.cores[c].tensor(k)[:] = v
        sim.simulate()
        results = [
            {"outT": np.array(sim.cores[c].tensor("outT"), np.float32)}
            for c in range(n_sim)
        ] + [
            {"outT": np.zeros((S, S), np.float32)} for _ in range(N_CORES - n_sim)
        ]
        LAST_RUN_INFO["exec_time_ns"] = None
    else:
        results = _get_runner(nc)(in_maps)
        LAST_RUN_INFO["exec_time_ns"] = None

    LAST_RUN_INFO["results"] = results
    return _reduce(results)
